# revision 1
# baseline (speedup 1.0000x reference)
"""Distributed causal multi-head attention (GPT-2 style block) for one TRN2 chip.

Sharding over 8 NeuronCores: core c -> (batch b = c//2, head-group g = c%2).
Each core computes QKV for its batch restricted to its 8 heads (tensor-
parallel column split of W_attn) and runs causal attention for those heads.
The pair of cores sharing a batch AllGathers the per-head attention output
(bf16, split in two halves per q-chunk so the collective overlaps attention),
then each core contracts the FULL 1024 head features against its own 512
columns of W_proj, producing a disjoint f32 slice of the output — no
reduce, no post-collective conversion step.

Attention inner loop processes HEAD PAIRS: the even head's K tile lives on
SBUF partitions 0-63 and the odd head's on 64-127, so their score matmuls
(contraction K=64 each) occupy disjoint PE row-groups and execute
concurrently (auto tile_position from base_partition).  One exp ACTIVATE
[128,1024] covers both heads' scores for a k-tile.  Softmax normalization
broadcasts 1/den across partitions with a tiny K=1 PE matmul into the
unused upper half of the PV PSUM bank (no GpSimd in the loop; GpSimd runs
only the collectives + their staging DMAs).

QKV matmul units for the NEXT q-chunk and output-projection units for
chunk qc-2 are injected between attention k-tiles as PE filler work, so the
PE never idles long enough for the HAM clock-gate to re-throttle.  Startup
interleaves the chunk-0 x loads with the V-weight columns on the SP HWDGE
ring while the Q/K weight columns and constants stream on the ACT ring.

Matmul dtypes: f32r (full-rate fp32 at N=512, ~1e-4 rel err) for QKV +
scores, bf16 for exp(P)/V and the output projection.  Softmax runs without
max-subtraction (logits are bounded), with the denominator computed by
augmenting V with a ones column so P@[V|1] yields numerator + denominator.
"""
import numpy as np
import ml_dtypes

B, S, D = 4, 2048, 1024
H, HD, HPC = 16, 64, 8
DL = HPC * HD            # 512 local head features / local out columns
P = 128
CW = 512                 # q-chunk width
NQC = S // CW            # 4
NKT = S // P             # 16
KC = D // P              # 8 contraction chunks of 128
VW = 65                  # per-head V width incl. ones column
MW = 384 + CW            # compacted causal mask width

_CACHE: dict = {}


def _build(debug=False):
    from concourse import bacc
    import concourse.mybir as mybir
    from concourse.tile import TileContext, add_dep_helper

    F32, F32R, BF16 = mybir.dt.float32, mybir.dt.float32r, mybir.dt.bfloat16
    AF = mybir.ActivationFunctionType
    ALU = mybir.AluOpType

    nc = bacc.Bacc(trn_type="TRN2", num_devices=8)
    if debug:
        dbg_vpad = nc.declare_dram_parameter(
            "dbg_vpad", [P, NKT, HPC * VW], BF16, isOutput=True)
        dbg_rc = nc.declare_dram_parameter(
            "dbg_rc", [NQC * HPC, CW], F32, isOutput=True)
        dbg_at = nc.declare_dram_parameter(
            "dbg_at", [4, P, CW], BF16, isOutput=True)
        dbg_agt = nc.declare_dram_parameter(
            "dbg_agt", [P, KC, CW], BF16, isOutput=True)
        dbg_qt = nc.declare_dram_parameter(
            "dbg_qt", [P, 4, CW], F32R, isOutput=True)
        dbg_kall = nc.declare_dram_parameter(
            "dbg_kall", [P, 4, S], F32R, isOutput=True)
    xT = nc.declare_dram_parameter("xT", [D, S], F32R, isOutput=False)
    wqkv = nc.declare_dram_parameter("wqkv", [D, 3 * DL], F32R, isOutput=False)
    bqk = nc.declare_dram_parameter("bqk", [P, 8], F32, isOutput=False)
    bv = nc.declare_dram_parameter("bv", [1, DL], F32, isOutput=False)
    wp = nc.declare_dram_parameter("wp", [D, DL], BF16, isOutput=False)
    bp = nc.declare_dram_parameter("bp", [P, 4], F32, isOutput=False)
    maskc = nc.declare_dram_parameter("maskc", [P, MW], BF16, isOutput=False)
    out_ext = nc.declare_dram_parameter("out", [DL, S], F32, isOutput=True)

    # per-chunk collective buffers; separate tensors avoid whole-tensor WAR
    # serialization across chunks.  Chunks 0-2 gather all 4 at tiles in one
    # AG; the last chunk splits in half so the first half overlaps attention.
    ag_in = [nc.dram_tensor(f"ag_in_{qc}", [4 * P, CW], BF16)
             for qc in range(NQC - 1)]
    ag_out = [nc.dram_tensor(f"ag_out_{qc}", [8 * P, CW], BF16)
              for qc in range(NQC - 1)]
    ag_in3 = [nc.dram_tensor(f"ag_in_3_{h}", [2 * P, CW], BF16)
              for h in range(2)]
    ag_out3 = [nc.dram_tensor(f"ag_out_3_{h}", [4 * P, CW], BF16)
               for h in range(2)]
    RG = [[0, 1], [2, 3], [4, 5], [6, 7]]

    with TileContext(nc) as tc:
        with tc.tile_pool(name="const", bufs=1) as constp, \
             tc.tile_pool(name="persist", bufs=1) as perp, \
             tc.tile_pool(name="wq", bufs=1) as wqp, \
             tc.tile_pool(name="xt", bufs=2) as xtp, \
             tc.tile_pool(name="qtp", bufs=3) as qtp, \
             tc.tile_pool(name="wpp", bufs=1) as wpp, \
             tc.tile_pool(name="ptp", bufs=3) as ptp, \
             tc.tile_pool(name="atp", bufs=1) as atp, \
             tc.tile_pool(name="smallp", bufs=1) as smallp, \
             tc.tile_pool(name="agtp", bufs=2) as agtp, \
             tc.tile_pool(name="otp", bufs=2) as otp, \
             tc.tile_pool(name="otap", bufs=4) as otap, \
             tc.tile_pool(name="ps3", bufs=2, space="PSUM") as ps3, \
             tc.tile_pool(name="ps1", bufs=3, space="PSUM") as ps1, \
             tc.tile_pool(name="psq", bufs=1, space="PSUM") as psq:

            # ---- chunk-0 x alone on the SP ring; V-weight columns head the
            # ACT ring, so both streams land kc-by-kc in parallel and the
            # first v_unit matmuls unblock at ~3us ----
            wq_t = wqp.tile([P, KC, 3 * DL], F32R)
            xtr0 = xtp.tile([P, KC, CW], F32R, tag="xtr", name="xtr_0")
            # The GpSimd FIFO must stay in program order: a staging DMA that
            # waits on an AllGather, scheduled ahead of a softmax broadcast,
            # head-of-line-blocks the whole attention pipeline.  Chain every
            # gpsimd-engine instruction to the previous one (collective
            # triggers live on the separate Collectives proc and are NOT
            # chained — waiting on them would serialize chunks).
            _gp_prev = [None]

            def gp(inst):
                if _gp_prev[0] is not None:
                    add_dep_helper(inst.ins, _gp_prev[0].ins,
                                   reason="gpsimd program order")
                _gp_prev[0] = inst
                return inst

            bv_stage = constp.tile([1, DL], F32)
            nc.scalar.dma_start(out=bv_stage[:], in_=bv[:])
            for kc in range(KC):
                nc.sync.dma_start(out=xtr0[:, kc, :],
                                  in_=xT[kc * P:(kc + 1) * P, 0:CW])
                nc.scalar.dma_start(out=wq_t[:, kc, 2 * DL:3 * DL],
                                    in_=wqkv[kc * P:(kc + 1) * P,
                                             2 * DL:3 * DL])

            # q-weight columns follow x on the SP ring; k-weight columns,
            # constants and wp follow the v columns on the ACT ring.
            for kc in range(KC):
                nc.sync.dma_start(out=wq_t[:, kc, 0:DL],
                                  in_=wqkv[kc * P:(kc + 1) * P, 0:DL])
            bqk_t = constp.tile([P, 8], F32)
            nc.scalar.dma_start(out=bqk_t[:], in_=bqk[:])
            bp_t = constp.tile([P, 4], F32)
            nc.scalar.dma_start(out=bp_t[:], in_=bp[:])
            maskr = constp.tile([P, MW], BF16)
            nc.scalar.dma_start(out=maskr[:], in_=maskc[:])
            bias_bc = constp.tile([P, DL], F32)
            gp(nc.gpsimd.partition_broadcast(bias_bc[:], bv_stage[:]))

            # ---- long-lived activations ----
            k_all = perp.tile([P, 4, S], F32R)
            vpad = perp.tile([P, NKT, HPC * VW], BF16)   # v + ones col per head
            ones_sb = constp.tile([P, NKT * HPC], BF16)
            nc.vector.memset(ones_sb[:], 1.0)
            nc.vector.tensor_copy(
                out=vpad[:].rearrange("p nk (h c) -> p (nk h) c",
                                      c=VW)[:, :, HD:VW],
                in_=ones_sb[:].unsqueeze(2))

            for kc in range(KC):
                nc.scalar.dma_start(out=wq_t[:, kc, DL:2 * DL],
                                    in_=wqkv[kc * P:(kc + 1) * P, DL:2 * DL])
            wp_t = wpp.tile([P, KC, DL], BF16)
            for kc in range(KC):
                nc.scalar.dma_start(out=wp_t[:, kc, :],
                                    in_=wp[kc * P:(kc + 1) * P, :])

            qt_tiles = {}
            at_sets = {}

            def qkv_units(qc, xtr=None, pool=None, ptag="psq"):
                """One generator item = one PE unit (8 matmuls + eviction).
                The upfront chunks use the (then idle) 3-deep ps1 pool so
                consecutive units pipeline past their evictions."""
                if pool is None:
                    pool = psq
                if xtr is None:
                    xtr = xtp.tile([P, KC, CW], F32R, tag="xtr",
                                   name=f"xtr_{qc}")
                    for kc in range(KC):
                        nc.sync.dma_start(
                            out=xtr[:, kc, :],
                            in_=xT[kc * P:(kc + 1) * P, qc * CW:(qc + 1) * CW])
                qt = qtp.tile([P, 4, CW], F32R, tag="qt", name=f"qt_{qc}")
                qt_tiles[qc] = qt

                def v_unit(stl):
                    pt = pool.tile([P, CW], F32, tag=ptag, name=f"v_{qc}_{stl}")
                    for kc in range(KC):
                        nc.tensor.matmul(
                            out=pt[:],
                            lhsT=xtr[:, kc, stl * P:(stl + 1) * P],
                            rhs=wq_t[:, kc, 2 * DL:3 * DL],
                            start=(kc == 0), stop=(kc == KC - 1))
                    st = qc * 4 + stl
                    nc.vector.tensor_tensor(
                        out=vpad[:, st, :].rearrange(
                            "p (h c) -> p h c", c=VW)[:, :, 0:HD],
                        in0=pt[:].rearrange("p (h c) -> p h c", c=HD),
                        in1=bias_bc[:].rearrange("p (h c) -> p h c", c=HD),
                        op=ALU.add)

                def qk_unit(m):
                    pt = pool.tile([P, CW], F32, tag=ptag, name=f"qk_{qc}_{m}")
                    for kc in range(KC):
                        nc.tensor.matmul(
                            out=pt[:],
                            lhsT=wq_t[:, kc, m * P:(m + 1) * P],
                            rhs=xtr[:, kc, :],
                            start=(kc == 0), stop=(kc == KC - 1))
                    dst = (qt[:, m, :] if m < 4
                           else k_all[:, m - 4, qc * CW:(qc + 1) * CW])
                    nc.vector.tensor_scalar_add(
                        out=dst, in0=pt[:], scalar1=bqk_t[:, m:m + 1])

                for stl in range(4):
                    yield lambda stl=stl: v_unit(stl)
                for m in range(8):
                    yield lambda m=m: qk_unit(m)

            agt_tiles = {}

            def stage_proj(qc):
                """Pull the pair-AllGathered at tiles back into SBUF.  On the
                gpsimd queue, scheduled mid-next-chunk so the AG has landed."""
                agt = agtp.tile([P, KC, CW], BF16, tag="agt", name=f"agt_{qc}")
                agt_tiles[qc] = agt
                if qc < NQC - 1:
                    gp(nc.gpsimd.dma_start(
                        out=agt[:, 0:4, :],
                        in_=ag_out[qc][0:4 * P, :].rearrange(
                            "(k p) s -> p k s", p=P)))
                    gp(nc.gpsimd.dma_start(
                        out=agt[:, 4:8, :],
                        in_=ag_out[qc][4 * P:8 * P, :].rearrange(
                            "(k p) s -> p k s", p=P)))
                if debug and qc == 0:
                    nc.sync.dma_start(out=dbg_agt[:], in_=agt[:])

            def proj_units(qc):
                """Full-feature projection onto this core's 512 out columns:
                4 oc-units of 8 matmuls each, from the AllGathered at tiles."""

                def unit(oc):
                    agt = agt_tiles[qc]
                    pp = psq.tile([P, CW], F32, tag="psq", name=f"pp_{qc}_{oc}")
                    for kc in range(KC):
                        nc.tensor.matmul(
                            out=pp[:],
                            lhsT=wp_t[:, kc, oc * P:(oc + 1) * P],
                            rhs=agt[:, kc, :],
                            start=(kc == 0), stop=(kc == KC - 1))
                    ot = otp.tile([P, CW], F32, tag="ot", name=f"ot_{qc}_{oc}")
                    nc.vector.tensor_scalar_add(out=ot[:], in0=pp[:],
                                                scalar1=bp_t[:, oc:oc + 1])
                    nc.sync.dma_start(
                        out=out_ext[oc * P:(oc + 1) * P,
                                    qc * CW:(qc + 1) * CW],
                        in_=ot[:])

                for oc in range(4):
                    yield lambda oc=oc: unit(oc)

            # ---- last-chunk projection, split so the half fed by the
            # early AllGather runs as attention filler and only the other
            # half trails the final collective ----
            agt3 = {}
            ota_tiles = {}

            def stage3_h0():
                agt = agtp.tile([P, KC, CW], BF16, tag="agt", name="agt_3")
                agt3["t"] = agt
                gp(nc.gpsimd.dma_start(
                    out=agt[:, 0:2, :],
                    in_=ag_out3[0][0:2 * P, :].rearrange("(k p) s -> p k s",
                                                         p=P)))
                gp(nc.gpsimd.dma_start(
                    out=agt[:, 4:6, :],
                    in_=ag_out3[0][2 * P:4 * P, :].rearrange(
                        "(k p) s -> p k s", p=P)))

            def stage3_h1():
                # tail-only: the sync ring is idle here and HWDGE has lower
                # first-byte latency than the SWDGE path.
                agt = agt3["t"]
                nc.sync.dma_start(
                    out=agt[:, 2:4, :],
                    in_=ag_out3[1][0:2 * P, :].rearrange("(k p) s -> p k s",
                                                         p=P))
                nc.sync.dma_start(
                    out=agt[:, 6:8, :],
                    in_=ag_out3[1][2 * P:4 * P, :].rearrange(
                        "(k p) s -> p k s", p=P))

            def proj3A_units():
                def unitA(oc):
                    agt = agt3["t"]
                    pp = psq.tile([P, CW], F32, tag="psq", name=f"ppA_{oc}")
                    for j, kc in enumerate((0, 1, 4, 5)):
                        nc.tensor.matmul(
                            out=pp[:],
                            lhsT=wp_t[:, kc, oc * P:(oc + 1) * P],
                            rhs=agt[:, kc, :],
                            start=(j == 0), stop=(j == 3))
                    ota = otap.tile([P, CW], BF16, tag="ota", name=f"ota_{oc}")
                    ota_tiles[oc] = ota
                    nc.vector.tensor_scalar_add(out=ota[:], in0=pp[:],
                                                scalar1=bp_t[:, oc:oc + 1])

                for oc in range(4):
                    yield lambda oc=oc: unitA(oc)

            def proj3B_units():
                def unitB(oc):
                    agt = agt3["t"]
                    # ps1's pa slots are free at the tail; 3-deep buffering
                    # lets the B matmul groups pipeline past their evictions.
                    pp = ps1.tile([P, CW], F32, tag="pa", name=f"ppB_{oc}")
                    for j, kc in enumerate((2, 3, 6, 7)):
                        nc.tensor.matmul(
                            out=pp[:],
                            lhsT=wp_t[:, kc, oc * P:(oc + 1) * P],
                            rhs=agt[:, kc, :],
                            start=(j == 0), stop=(j == 3))
                    ot = otp.tile([P, CW], F32, tag="ot", name=f"otB_{oc}")
                    nc.vector.tensor_tensor(out=ot[:], in0=pp[:],
                                            in1=ota_tiles[oc][:], op=ALU.add)
                    nc.sync.dma_start(
                        out=out_ext[oc * P:(oc + 1) * P,
                                    (NQC - 1) * CW:NQC * CW],
                        in_=ot[:])

                for oc in range(4):
                    yield lambda oc=oc: unitB(oc)

            fillers = []
            fillers_late = []

            def pop_filler(late_ok):
                # keep 2 units in reserve: drain_fillers emits them after the
                # chunk's attention so the PE stays busy (and HAM warm) across
                # the chunk boundary while the last pair's softmax normalize
                # chain releases its PSUM slots.
                if len(fillers) > 2:
                    fillers.pop(0)()
                elif late_ok and fillers_late:
                    fillers_late.pop(0)()

            def drain_fillers():
                while fillers:
                    fillers.pop(0)()
                while fillers_late:
                    fillers_late.pop(0)()

            def emit_attention(qc, stage_cb=None, late_cb=None):
                qt = qt_tiles[qc]
                at_tiles = [None] * 4
                at_sets[qc] = at_tiles
                kmax = 4 * (qc + 1)

                for hp in range(4):
                    if hp == 2 and stage_cb is not None:
                        stage_cb()
                    if hp == 3 and late_cb is not None:
                        late_cb()
                    h_e, h_o = 2 * hp, 2 * hp + 1
                    qs_e = qt[0:64, hp, :]
                    qs_o = qt[64:128, hp, :]
                    pa_e = ps1.tile([P, CW], F32, tag="pa",
                                    name=f"pa_{qc}_{hp}_e")
                    pa_o = ps1.tile([P, CW], F32, tag="pa",
                                    name=f"pa_{qc}_{hp}_o")
                    pending = None

                    def flush(pending, kmax=kmax, pa_e=pa_e, pa_o=pa_o,
                              h_e=h_e, h_o=h_o, qc=qc):
                        kt, ptile = pending
                        if kt >= 4 * qc:
                            off = 384 - (kt - 4 * qc) * P
                            nc.vector.tensor_mul(
                                out=ptile[:, 0:CW], in0=ptile[:, 0:CW],
                                in1=maskr[:, off:off + CW])
                            nc.vector.tensor_mul(
                                out=ptile[:, CW:2 * CW], in0=ptile[:, CW:2 * CW],
                                in1=maskr[:, off:off + CW])
                        nc.tensor.matmul(
                            out=pa_e[0:VW, :],
                            lhsT=vpad[:, kt, h_e * VW:(h_e + 1) * VW],
                            rhs=ptile[:, 0:CW],
                            start=(kt == 0), stop=(kt == kmax - 1))
                        nc.tensor.matmul(
                            out=pa_o[0:VW, :],
                            lhsT=vpad[:, kt, h_o * VW:(h_o + 1) * VW],
                            rhs=ptile[:, CW:2 * CW],
                            start=(kt == 0), stop=(kt == kmax - 1))

                    for kt in range(kmax):
                        pt = ps3.tile([P, 2 * CW], F32, tag="ps3",
                                      name=f"sc_{qc}_{hp}_{kt}")
                        nc.tensor.matmul(
                            out=pt[:, 0:CW],
                            lhsT=k_all[0:64, hp, kt * P:(kt + 1) * P],
                            rhs=qs_e, start=True, stop=True)
                        nc.tensor.matmul(
                            out=pt[:, CW:2 * CW],
                            lhsT=k_all[64:128, hp, kt * P:(kt + 1) * P],
                            rhs=qs_o, start=True, stop=True)
                        if pending is not None:
                            flush(pending)
                        ptile = ptp.tile([P, 2 * CW], BF16, tag="pt",
                                         name=f"pt_{qc}_{hp}_{kt}")
                        nc.scalar.activation(ptile[:], pt[:], AF.Exp,
                                             scale=0.125)
                        pending = (kt, ptile)
                        pop_filler(late_ok=(hp == 3 and kt >= kmax - 10))
                    flush(pending)

                    # normalize by the ones-row denominator
                    at = atp.tile([P, CW], BF16, tag=f"at{hp}",
                                  name=f"at_{qc}_{hp}")
                    at_tiles[hp] = at
                    for half, pa in ((0, pa_e), (64, pa_o)):
                        # den must bounce PSUM->SBUF: custom-DVE ops misread
                        # PSUM APs with nonzero base_partition on HW.
                        den = smallp.tile([1, CW], F32, tag="den",
                                          name=f"den_{qc}_{hp}_{half}")
                        nc.vector.tensor_copy(out=den[:], in_=pa[64:65, :])
                        rc = smallp.tile([1, CW], F32, tag="recip",
                                         name=f"rc_{qc}_{hp}_{half}")
                        nc.vector.reciprocal_approx_fast(
                            out=rc[:], in_=den[:])
                        if debug:
                            hidx = qc * HPC + hp * 2 + (half // 64)
                            nc.sync.dma_start(
                                out=dbg_rc[hidx:hidx + 1, :], in_=rc[:])
                        bc = smallp.tile([64, CW], F32, tag="bcast",
                                         name=f"bc_{qc}_{hp}_{half}")
                        gp(nc.gpsimd.partition_broadcast(bc[:], rc[:]))
                        nc.vector.tensor_tensor(
                            out=at[half:half + 64, :],
                            in0=pa[0:64, :], in1=bc[:], op=ALU.mult)
                    if debug and qc == 0:
                        nc.sync.dma_start(out=dbg_at[hp, :, :], in_=at[:])
                    # ship this pair's at rows toward the pair AllGather
                    if qc < NQC - 1:
                        nc.sync.dma_start(
                            out=ag_in[qc][hp * P:(hp + 1) * P, :], in_=at[:])
                        if hp == 3:
                            nc.gpsimd.collective_compute(
                                "AllGather", ALU.bypass, replica_groups=RG,
                                ins=[ag_in[qc][:]], outs=[ag_out[qc][:]])
                    else:
                        nc.sync.dma_start(
                            out=ag_in3[hp // 2][(hp % 2) * P:
                                               (hp % 2 + 1) * P, :],
                            in_=at[:])
                        if hp % 2 == 1:
                            nc.gpsimd.collective_compute(
                                "AllGather", ALU.bypass, replica_groups=RG,
                                ins=[ag_in3[hp // 2][:]],
                                outs=[ag_out3[hp // 2][:]])

            # ---- main schedule ----
            # chunk 0 QKV up front; chunk qc+1's QKV rides as PE filler in
            # attention(qc); proj(qc) rides in attention(qc+2) (its pair
            # AllGather completed during attention(qc+1)); proj(2) rides in
            # the second half of attention(3); proj(3) is the tail.
            for u in qkv_units(0, xtr=xtr0):
                u()
            if debug:
                nc.sync.dma_start(out=dbg_qt[:], in_=qt_tiles[0][:])
            # chunk-1 QKV also runs up front: its DMAs stream behind chunk
            # 0's on both rings while the PE chews through chunk-0 units, so
            # the start is PE-dense.  Attention chunks then host qkv(qc+2)
            # and the projections concentrate in attention(3), whose causal
            # k-range leaves the most ACT-paced slack to fill.
            for u in qkv_units(1):
                u()
            held = []
            for qc in range(NQC):
                if qc + 2 < NQC:
                    units = list(qkv_units(qc + 2))
                    if qc == 1:
                        fillers.extend(units[:6])
                        held = units[6:]
                    else:
                        fillers.extend(units)
                if qc == 2:
                    fillers.extend(held)
                    held = []
                    fillers.extend(proj_units(0))
                if qc == NQC - 1:
                    fillers.extend(proj_units(1))
                    fillers_late.extend(proj_units(2))
                    fillers_late.extend(proj3A_units())
                emit_attention(
                    qc,
                    stage_cb=((lambda q=qc: stage_proj(q - 1)) if qc >= 1
                              else None),
                    late_cb=(stage3_h0 if qc == NQC - 1 else None))
                drain_fillers()
            stage3_h1()
            for u in proj3B_units():
                u()
            if debug:
                nc.sync.dma_start(out=dbg_vpad[:], in_=vpad[:])
                nc.sync.dma_start(out=dbg_kall[:], in_=k_all[:])

    nc.finalize()
    return nc


def _get_nc():
    if "nc" not in _CACHE:
        _CACHE["nc"] = _build()
    return _CACHE["nc"]


def _make_mask():
    # M[p, c] = 1.0 iff (c - 384) >= p; pattern pat slice = cols [384-128*pat:][:CW]
    c = np.arange(MW)[None, :]
    p = np.arange(P)[:, None]
    return ((c - 384) >= p).astype(ml_dtypes.bfloat16)


def make_in_maps(x, W_attn, b_attn, W_proj, b_proj):
    x = np.asarray(x, np.float32)
    W_attn = np.asarray(W_attn, np.float32)
    b_attn = np.asarray(b_attn, np.float32)
    W_proj = np.asarray(W_proj, np.float32)
    b_proj = np.asarray(b_proj, np.float32)
    mask = _make_mask()
    in_maps = []
    for c in range(8):
        b, g = c // 2, c % 2
        sl = slice(g * DL, (g + 1) * DL)
        wqkv_c = np.concatenate([W_attn[:, g * DL:(g + 1) * DL],
                                 W_attn[:, D + g * DL:D + (g + 1) * DL],
                                 W_attn[:, 2 * D + g * DL:2 * D + (g + 1) * DL]],
                                axis=1)
        bqk_c = np.concatenate([b_attn[g * DL:(g + 1) * DL],
                                b_attn[D + g * DL:D + (g + 1) * DL]])
        in_maps.append({
            "xT": np.ascontiguousarray(x[b].T),
            "wqkv": np.ascontiguousarray(wqkv_c),
            "bqk": np.ascontiguousarray(bqk_c.reshape(8, P).T),
            "bv": b_attn[2 * D + g * DL:2 * D + (g + 1) * DL].reshape(1, DL).copy(),
            "wp": np.ascontiguousarray(W_proj[:, sl]).astype(ml_dtypes.bfloat16),
            "bp": np.ascontiguousarray(b_proj[sl].reshape(4, P).T),
            "maskc": mask,
        })
    return in_maps


def assemble(results):
    out = np.empty((B, S, D), np.float32)
    for c in range(8):
        b, g = c // 2, c % 2
        out[b][:, g * DL:(g + 1) * DL] = results[c]["out"].T
    return out


def kernel(x, W_attn, b_attn, W_proj, b_proj):
    from concourse.bass_utils import run_bass_kernel_spmd
    nc = _get_nc()
    in_maps = make_in_maps(x, W_attn, b_attn, W_proj, b_proj)
    res = run_bass_kernel_spmd(nc, in_maps, core_ids=list(range(8)))
    return assemble(res.results)



# revision 13
# speedup vs baseline: 1.0273x; 1.0273x over previous
"""Distributed causal multi-head attention (GPT-2 style block) for one TRN2 chip.

Sharding over 8 NeuronCores: core c -> (batch b = c//2, head-group g = c%2).
Each core computes QKV for its batch restricted to its 8 heads (tensor-
parallel column split of W_attn) and runs causal attention for those heads.
The pair of cores sharing a batch AllGathers the per-head attention output
(bf16, split in two halves per q-chunk so the collective overlaps attention),
then each core contracts the FULL 1024 head features against its own 512
columns of W_proj, producing a disjoint f32 slice of the output — no
reduce, no post-collective conversion step.

Attention inner loop processes HEAD PAIRS: the even head's K tile lives on
SBUF partitions 0-63 and the odd head's on 64-127, so their score matmuls
(contraction K=64 each) occupy disjoint PE row-groups and execute
concurrently (auto tile_position from base_partition).  One exp ACTIVATE
[128,1024] covers both heads' scores for a k-tile.  Softmax normalization
broadcasts 1/den across partitions with a tiny K=1 PE matmul into the
unused upper half of the PV PSUM bank (no GpSimd in the loop; GpSimd runs
only the collectives + their staging DMAs).

QKV matmul units for the NEXT q-chunk and output-projection units for
chunk qc-2 are injected between attention k-tiles as PE filler work, so the
PE never idles long enough for the HAM clock-gate to re-throttle.  Startup
interleaves the chunk-0 x loads with the V-weight columns on the SP HWDGE
ring while the Q/K weight columns and constants stream on the ACT ring.

Matmul dtypes: f32r (full-rate fp32 at N=512, ~1e-4 rel err) for QKV +
scores, bf16 for exp(P)/V and the output projection.  Softmax runs without
max-subtraction (logits are bounded), with the denominator computed by
augmenting V with a ones column so P@[V|1] yields numerator + denominator.
"""
import numpy as np
import ml_dtypes

B, S, D = 4, 2048, 1024
H, HD, HPC = 16, 64, 8
DL = HPC * HD            # 512 local head features / local out columns
P = 128
CW = 512                 # q-chunk width
NQC = S // CW            # 4
NKT = S // P             # 16
KC = D // P              # 8 contraction chunks of 128
VW = 65                  # per-head V width incl. ones column
MW = 384 + CW            # compacted causal mask width

_CACHE: dict = {}


def _build(debug=False):
    from concourse import bacc
    import concourse.mybir as mybir
    from concourse.tile import TileContext, add_dep_helper

    F32, F32R, BF16 = mybir.dt.float32, mybir.dt.float32r, mybir.dt.bfloat16
    AF = mybir.ActivationFunctionType
    ALU = mybir.AluOpType

    nc = bacc.Bacc(trn_type="TRN2", num_devices=8)
    if debug:
        dbg_vpad = nc.declare_dram_parameter(
            "dbg_vpad", [P, NKT, HPC * VW], BF16, isOutput=True)
        dbg_rc = nc.declare_dram_parameter(
            "dbg_rc", [NQC * HPC, CW], F32, isOutput=True)
        dbg_at = nc.declare_dram_parameter(
            "dbg_at", [4, P, CW], BF16, isOutput=True)
        dbg_agt = nc.declare_dram_parameter(
            "dbg_agt", [P, KC, CW], BF16, isOutput=True)
        dbg_qt = nc.declare_dram_parameter(
            "dbg_qt", [P, 4, CW], BF16, isOutput=True)
        dbg_kall = nc.declare_dram_parameter(
            "dbg_kall", [P, 4, S], BF16, isOutput=True)
    xT = nc.declare_dram_parameter("xT", [D, S], BF16, isOutput=False)
    wqkv = nc.declare_dram_parameter("wqkv", [D, 3 * DL], BF16, isOutput=False)
    bqk = nc.declare_dram_parameter("bqk", [P, 8], F32, isOutput=False)
    bv = nc.declare_dram_parameter("bv", [1, DL], F32, isOutput=False)
    wp = nc.declare_dram_parameter("wp", [D, DL], BF16, isOutput=False)
    bp = nc.declare_dram_parameter("bp", [P, 4], F32, isOutput=False)
    maskc = nc.declare_dram_parameter("maskc", [P, MW], BF16, isOutput=False)
    out_ext = nc.declare_dram_parameter("out", [DL, S], F32, isOutput=True)

    # per-chunk collective buffers; separate tensors avoid whole-tensor WAR
    # serialization across chunks.  Chunks 0-2 gather all 4 at tiles in one
    # AG; the last chunk splits in half so the first half overlaps attention.
    ag_in = [nc.dram_tensor(f"ag_in_{qc}", [4 * P, CW], BF16)
             for qc in range(NQC - 1)]
    ag_out = [nc.dram_tensor(f"ag_out_{qc}", [8 * P, CW], BF16)
              for qc in range(NQC - 1)]
    # last chunk: three collectives of decreasing size so the final exposed
    # one carries only a single head-pair (128 KB, ~6us latency).
    ag_in3A = nc.dram_tensor("ag_in_3A", [2 * P, CW], BF16)
    ag_out3A = nc.dram_tensor("ag_out_3A", [4 * P, CW], BF16)
    ag_in3B = nc.dram_tensor("ag_in_3B", [P, CW], BF16)
    ag_out3B = nc.dram_tensor("ag_out_3B", [2 * P, CW], BF16)
    ag_in3C = nc.dram_tensor("ag_in_3C", [P, CW], BF16)
    ag_out3C = nc.dram_tensor("ag_out_3C", [2 * P, CW], BF16)
    RG = [[0, 1], [2, 3], [4, 5], [6, 7]]

    with TileContext(nc) as tc:
        with tc.tile_pool(name="const", bufs=1) as constp, \
             tc.tile_pool(name="persist", bufs=1) as perp, \
             tc.tile_pool(name="wq", bufs=1) as wqp, \
             tc.tile_pool(name="xt", bufs=2) as xtp, \
             tc.tile_pool(name="qtp", bufs=3) as qtp, \
             tc.tile_pool(name="wpp", bufs=1) as wpp, \
             tc.tile_pool(name="ptp", bufs=3) as ptp, \
             tc.tile_pool(name="atp", bufs=1) as atp, \
             tc.tile_pool(name="smallp", bufs=1) as smallp, \
             tc.tile_pool(name="agtp", bufs=2) as agtp, \
             tc.tile_pool(name="otp", bufs=2) as otp, \
             tc.tile_pool(name="otap", bufs=8) as otap, \
             tc.tile_pool(name="ps3", bufs=2, space="PSUM") as ps3, \
             tc.tile_pool(name="ps1", bufs=3, space="PSUM") as ps1, \
             tc.tile_pool(name="psq", bufs=1, space="PSUM") as psq:

            # ---- chunk-0 x alone on the SP ring; V-weight columns head the
            # ACT ring, so both streams land kc-by-kc in parallel and the
            # first v_unit matmuls unblock at ~3us ----
            wq_t = wqp.tile([P, KC, 3 * DL], BF16)
            xtr0 = xtp.tile([P, KC, CW], BF16, tag="xtr", name="xtr_0")
            # The GpSimd FIFO must stay in program order: a staging DMA that
            # waits on an AllGather, scheduled ahead of a softmax broadcast,
            # head-of-line-blocks the whole attention pipeline.  Chain every
            # gpsimd-engine instruction to the previous one (collective
            # triggers live on the separate Collectives proc and are NOT
            # chained — waiting on them would serialize chunks).
            _gp_prev = [None]

            def gp(inst):
                if _gp_prev[0] is not None:
                    add_dep_helper(inst.ins, _gp_prev[0].ins,
                                   reason="gpsimd program order")
                _gp_prev[0] = inst
                return inst

            bv_stage = constp.tile([1, DL], F32)
            nc.scalar.dma_start(out=bv_stage[:], in_=bv[:])
            for kc in range(KC):
                nc.sync.dma_start(out=xtr0[:, kc, :],
                                  in_=xT[kc * P:(kc + 1) * P, 0:CW])
                nc.scalar.dma_start(out=wq_t[:, kc, 2 * DL:3 * DL],
                                    in_=wqkv[kc * P:(kc + 1) * P,
                                             2 * DL:3 * DL])

            # q-weight columns follow x on the SP ring; k-weight columns,
            # constants and wp follow the v columns on the ACT ring.
            for kc in range(KC):
                nc.sync.dma_start(out=wq_t[:, kc, 0:DL],
                                  in_=wqkv[kc * P:(kc + 1) * P, 0:DL])
            bqk_t = constp.tile([P, 8], F32)
            nc.scalar.dma_start(out=bqk_t[:], in_=bqk[:])
            bp_t = constp.tile([P, 4], F32)
            nc.scalar.dma_start(out=bp_t[:], in_=bp[:])
            maskr = constp.tile([P, MW], BF16)
            nc.scalar.dma_start(out=maskr[:], in_=maskc[:])
            bias_bc = constp.tile([P, DL], F32)
            gp(nc.gpsimd.partition_broadcast(bias_bc[:], bv_stage[:]))

            # ---- long-lived activations ----
            k_all = perp.tile([P, 4, S], BF16)
            vpad = perp.tile([P, NKT, HPC * VW], BF16)   # v + ones col per head
            ones_sb = constp.tile([P, NKT * HPC], BF16)
            nc.vector.memset(ones_sb[:], 1.0)
            nc.vector.tensor_copy(
                out=vpad[:].rearrange("p nk (h c) -> p (nk h) c",
                                      c=VW)[:, :, HD:VW],
                in_=ones_sb[:].unsqueeze(2))

            for kc in range(KC):
                nc.scalar.dma_start(out=wq_t[:, kc, DL:2 * DL],
                                    in_=wqkv[kc * P:(kc + 1) * P, DL:2 * DL])
            wp_t = wpp.tile([P, KC, DL], BF16)
            for kc in range(KC):
                nc.scalar.dma_start(out=wp_t[:, kc, :],
                                    in_=wp[kc * P:(kc + 1) * P, :])

            qt_tiles = {}
            at_sets = {}

            def qkv_units(qc, xtr=None, pool=None, ptag="psq"):
                """One generator item = one PE unit (8 matmuls + eviction).
                The upfront chunks use the (then idle) 3-deep ps1 pool so
                consecutive units pipeline past their evictions."""
                if pool is None:
                    pool = psq
                if xtr is None:
                    xtr = xtp.tile([P, KC, CW], BF16, tag="xtr",
                                   name=f"xtr_{qc}")
                    for kc in range(KC):
                        nc.sync.dma_start(
                            out=xtr[:, kc, :],
                            in_=xT[kc * P:(kc + 1) * P, qc * CW:(qc + 1) * CW])
                qt = qtp.tile([P, 4, CW], BF16, tag="qt", name=f"qt_{qc}")
                qt_tiles[qc] = qt

                def v_unit(stl):
                    pt = pool.tile([P, CW], F32, tag=ptag, name=f"v_{qc}_{stl}")
                    for kc in range(KC):
                        nc.tensor.matmul(
                            out=pt[:],
                            lhsT=xtr[:, kc, stl * P:(stl + 1) * P],
                            rhs=wq_t[:, kc, 2 * DL:3 * DL],
                            start=(kc == 0), stop=(kc == KC - 1))
                    st = qc * 4 + stl
                    nc.vector.tensor_tensor(
                        out=vpad[:, st, :].rearrange(
                            "p (h c) -> p h c", c=VW)[:, :, 0:HD],
                        in0=pt[:].rearrange("p (h c) -> p h c", c=HD),
                        in1=bias_bc[:].rearrange("p (h c) -> p h c", c=HD),
                        op=ALU.add)

                def qk_unit(m):
                    pt = pool.tile([P, CW], F32, tag=ptag, name=f"qk_{qc}_{m}")
                    for kc in range(KC):
                        nc.tensor.matmul(
                            out=pt[:],
                            lhsT=wq_t[:, kc, m * P:(m + 1) * P],
                            rhs=xtr[:, kc, :],
                            start=(kc == 0), stop=(kc == KC - 1))
                    dst = (qt[:, m, :] if m < 4
                           else k_all[:, m - 4, qc * CW:(qc + 1) * CW])
                    nc.vector.tensor_scalar_add(
                        out=dst, in0=pt[:], scalar1=bqk_t[:, m:m + 1])

                for stl in range(4):
                    yield lambda stl=stl: v_unit(stl)
                for m in range(8):
                    yield lambda m=m: qk_unit(m)

            agt_tiles = {}

            def stage_proj(qc):
                """Pull the pair-AllGathered at tiles back into SBUF.  On the
                gpsimd queue, scheduled mid-next-chunk so the AG has landed."""
                agt = agtp.tile([P, KC, CW], BF16, tag="agt", name=f"agt_{qc}")
                agt_tiles[qc] = agt
                if qc < NQC - 1:
                    gp(nc.gpsimd.dma_start(
                        out=agt[:, 0:4, :],
                        in_=ag_out[qc][0:4 * P, :].rearrange(
                            "(k p) s -> p k s", p=P)))
                    gp(nc.gpsimd.dma_start(
                        out=agt[:, 4:8, :],
                        in_=ag_out[qc][4 * P:8 * P, :].rearrange(
                            "(k p) s -> p k s", p=P)))
                if debug and qc == 0:
                    nc.sync.dma_start(out=dbg_agt[:], in_=agt[:])

            def proj_units(qc):
                """Full-feature projection onto this core's 512 out columns:
                4 oc-units of 8 matmuls each, from the AllGathered at tiles."""

                def unit(oc):
                    agt = agt_tiles[qc]
                    pp = psq.tile([P, CW], F32, tag="psq", name=f"pp_{qc}_{oc}")
                    for kc in range(KC):
                        nc.tensor.matmul(
                            out=pp[:],
                            lhsT=wp_t[:, kc, oc * P:(oc + 1) * P],
                            rhs=agt[:, kc, :],
                            start=(kc == 0), stop=(kc == KC - 1))
                    ot = otp.tile([P, CW], F32, tag="ot", name=f"ot_{qc}_{oc}")
                    nc.vector.tensor_scalar_add(out=ot[:], in0=pp[:],
                                                scalar1=bp_t[:, oc:oc + 1])
                    nc.sync.dma_start(
                        out=out_ext[oc * P:(oc + 1) * P,
                                    qc * CW:(qc + 1) * CW],
                        in_=ot[:])

                for oc in range(4):
                    yield lambda oc=oc: unit(oc)

            # ---- last-chunk projection, staged per collective: A (hp0+hp1,
            # gathered mid-chunk) runs as attention filler; B (hp2) right
            # after attention while the final C (hp3) collective flies; C
            # trails with just 8 matmuls and ring-parallel output DMAs ----
            agt3 = {}
            ota_tiles = {}
            ota2_tiles = {}

            def stage3_A():
                agt = agtp.tile([P, KC, CW], BF16, tag="agt", name="agt_3")
                agt3["t"] = agt
                gp(nc.gpsimd.dma_start(
                    out=agt[:, 0:2, :],
                    in_=ag_out3A[0:2 * P, :].rearrange("(k p) s -> p k s",
                                                       p=P)))
                gp(nc.gpsimd.dma_start(
                    out=agt[:, 4:6, :],
                    in_=ag_out3A[2 * P:4 * P, :].rearrange(
                        "(k p) s -> p k s", p=P)))

            def stage3_B():
                # tail-only: the sync ring is idle here and HWDGE has lower
                # first-byte latency than the SWDGE path.
                agt = agt3["t"]
                nc.sync.dma_start(out=agt[:, 2, :], in_=ag_out3B[0:P, :])
                nc.sync.dma_start(out=agt[:, 6, :], in_=ag_out3B[P:2 * P, :])

            def stage3_C():
                agt = agt3["t"]
                nc.sync.dma_start(out=agt[:, 3, :], in_=ag_out3C[0:P, :])
                nc.sync.dma_start(out=agt[:, 7, :], in_=ag_out3C[P:2 * P, :])

            def proj3A_units():
                def unitA(oc):
                    agt = agt3["t"]
                    pp = psq.tile([P, CW], F32, tag="psq", name=f"ppA_{oc}")
                    for j, kc in enumerate((0, 1, 4, 5)):
                        nc.tensor.matmul(
                            out=pp[:],
                            lhsT=wp_t[:, kc, oc * P:(oc + 1) * P],
                            rhs=agt[:, kc, :],
                            start=(j == 0), stop=(j == 3))
                    ota = otap.tile([P, CW], BF16, tag="ota", name=f"ota_{oc}")
                    ota_tiles[oc] = ota
                    nc.vector.tensor_scalar_add(out=ota[:], in0=pp[:],
                                                scalar1=bp_t[:, oc:oc + 1])

                for oc in range(4):
                    yield lambda oc=oc: unitA(oc)

            def proj3B_units():
                def unitB(oc):
                    agt = agt3["t"]
                    # ps1's pa slots are free at the tail; 3-deep buffering
                    # lets the B matmul groups pipeline past their evictions.
                    pp = ps1.tile([P, CW], F32, tag="pa", name=f"ppB_{oc}")
                    for j, kc in enumerate((2, 6)):
                        nc.tensor.matmul(
                            out=pp[:],
                            lhsT=wp_t[:, kc, oc * P:(oc + 1) * P],
                            rhs=agt[:, kc, :],
                            start=(j == 0), stop=(j == 1))
                    ota2 = otap.tile([P, CW], BF16, tag="ota2",
                                     name=f"ota2_{oc}")
                    ota2_tiles[oc] = ota2
                    nc.vector.tensor_tensor(out=ota2[:], in0=pp[:],
                                            in1=ota_tiles[oc][:], op=ALU.add)

                for oc in range(4):
                    yield lambda oc=oc: unitB(oc)

            def proj3C_units():
                def unitC(oc):
                    agt = agt3["t"]
                    pp = ps1.tile([P, CW], F32, tag="pa", name=f"ppC_{oc}")
                    for j, kc in enumerate((3, 7)):
                        nc.tensor.matmul(
                            out=pp[:],
                            lhsT=wp_t[:, kc, oc * P:(oc + 1) * P],
                            rhs=agt[:, kc, :],
                            start=(j == 0), stop=(j == 1))
                    ot = otp.tile([P, CW], F32, tag="ot", name=f"otC_{oc}")
                    nc.vector.tensor_tensor(out=ot[:], in0=pp[:],
                                            in1=ota2_tiles[oc][:], op=ALU.add)
                    dst = out_ext[oc * P:(oc + 1) * P,
                                  (NQC - 1) * CW:NQC * CW]
                    # spread the four final stores across engine rings so the
                    # kernel tail is one DMA deep, not four.
                    if oc == 0:
                        nc.sync.dma_start(out=dst, in_=ot[:])
                    elif oc == 1:
                        nc.scalar.dma_start(out=dst, in_=ot[:])
                    elif oc == 2:
                        gp(nc.gpsimd.dma_start(out=dst, in_=ot[:]))
                    else:
                        nc.sync.dma_start(out=dst, in_=ot[:])

                for oc in range(4):
                    yield lambda oc=oc: unitC(oc)

            fillers = []
            fillers_late = []

            def pop_filler(late_ok):
                # keep 2 units in reserve: drain_fillers emits them after the
                # chunk's attention so the PE stays busy (and HAM warm) across
                # the chunk boundary while the last pair's softmax normalize
                # chain releases its PSUM slots.
                if len(fillers) > 2:
                    fillers.pop(0)()
                elif late_ok and fillers_late:
                    fillers_late.pop(0)()

            def drain_fillers():
                while fillers:
                    fillers.pop(0)()
                while fillers_late:
                    fillers_late.pop(0)()

            def emit_attention(qc, stage_cb=None, late_cb=None):
                qt = qt_tiles[qc]
                at_tiles = [None] * 4
                at_sets[qc] = at_tiles
                kmax = 4 * (qc + 1)

                for hp in range(4):
                    if hp == 2 and stage_cb is not None:
                        stage_cb()
                    if hp == 3 and late_cb is not None:
                        late_cb()
                    h_e, h_o = 2 * hp, 2 * hp + 1
                    qs_e = qt[0:64, hp, :]
                    qs_o = qt[64:128, hp, :]
                    pa_e = ps1.tile([P, CW], F32, tag="pa",
                                    name=f"pa_{qc}_{hp}_e")
                    pa_o = ps1.tile([P, CW], F32, tag="pa",
                                    name=f"pa_{qc}_{hp}_o")
                    pending = None

                    # Diagonal k-tiles (kt >= 4qc) only have valid queries at
                    # columns >= 128*(kt-4qc): trim the score matmul, the exp
                    # ACTIVATE, the causal-mask multiplies and the PV matmuls
                    # to that column range.  The skipped columns are exactly
                    # the fully-masked region, so this is bit-equivalent.
                    def toff(kt):
                        return max(0, (kt - 4 * qc)) * P

                    def flush(pending, kmax=kmax, pa_e=pa_e, pa_o=pa_o,
                              h_e=h_e, h_o=h_o, qc=qc):
                        kt, ptile = pending
                        off = toff(kt)
                        if kt >= 4 * qc:
                            nc.vector.tensor_mul(
                                out=ptile[:, off:CW], in0=ptile[:, off:CW],
                                in1=maskr[:, 384:384 + CW - off])
                            nc.vector.tensor_mul(
                                out=ptile[:, CW + off:2 * CW],
                                in0=ptile[:, CW + off:2 * CW],
                                in1=maskr[:, 384:384 + CW - off])
                        nc.tensor.matmul(
                            out=pa_e[0:VW, off:CW],
                            lhsT=vpad[:, kt, h_e * VW:(h_e + 1) * VW],
                            rhs=ptile[:, off:CW],
                            start=(kt == 0), stop=(kt == kmax - 1))
                        nc.tensor.matmul(
                            out=pa_o[0:VW, off:CW],
                            lhsT=vpad[:, kt, h_o * VW:(h_o + 1) * VW],
                            rhs=ptile[:, CW + off:2 * CW],
                            start=(kt == 0), stop=(kt == kmax - 1))

                    for kt in range(kmax):
                        off = toff(kt)
                        pt = ps3.tile([P, 2 * CW], F32, tag="ps3",
                                      name=f"sc_{qc}_{hp}_{kt}")
                        nc.tensor.matmul(
                            out=pt[:, off:CW],
                            lhsT=k_all[0:64, hp, kt * P:(kt + 1) * P],
                            rhs=qt[0:64, hp, off:CW], start=True, stop=True)
                        nc.tensor.matmul(
                            out=pt[:, CW + off:2 * CW],
                            lhsT=k_all[64:128, hp, kt * P:(kt + 1) * P],
                            rhs=qt[64:128, hp, off:CW], start=True, stop=True)
                        if pending is not None:
                            flush(pending)
                        ptile = ptp.tile([P, 2 * CW], BF16, tag="pt",
                                         name=f"pt_{qc}_{hp}_{kt}")
                        if off:
                            nc.scalar.activation(
                                ptile[:].rearrange("p (h w) -> p h w",
                                                   w=CW)[:, :, off:CW],
                                pt[:].rearrange("p (h w) -> p h w",
                                                w=CW)[:, :, off:CW],
                                AF.Exp, scale=0.125)
                        else:
                            nc.scalar.activation(ptile[:], pt[:], AF.Exp,
                                                 scale=0.125)
                        pending = (kt, ptile)
                        pop_filler(late_ok=(hp == 3 and kt >= kmax - 10))
                    flush(pending)

                    # normalize by the ones-row denominator
                    at = atp.tile([P, CW], BF16, tag=f"at{hp}",
                                  name=f"at_{qc}_{hp}")
                    at_tiles[hp] = at
                    for half, pa in ((0, pa_e), (64, pa_o)):
                        # den must bounce PSUM->SBUF: custom-DVE ops misread
                        # PSUM APs with nonzero base_partition on HW.
                        den = smallp.tile([1, CW], F32, tag="den",
                                          name=f"den_{qc}_{hp}_{half}")
                        nc.vector.tensor_copy(out=den[:], in_=pa[64:65, :])
                        rc = smallp.tile([1, CW], F32, tag="recip",
                                         name=f"rc_{qc}_{hp}_{half}")
                        nc.vector.reciprocal_approx_fast(
                            out=rc[:], in_=den[:])
                        if debug:
                            hidx = qc * HPC + hp * 2 + (half // 64)
                            nc.sync.dma_start(
                                out=dbg_rc[hidx:hidx + 1, :], in_=rc[:])
                        bc = smallp.tile([64, CW], F32, tag="bcast",
                                         name=f"bc_{qc}_{hp}_{half}")
                        gp(nc.gpsimd.partition_broadcast(bc[:], rc[:]))
                        nc.vector.tensor_tensor(
                            out=at[half:half + 64, :],
                            in0=pa[0:64, :], in1=bc[:], op=ALU.mult)
                    if debug and qc == 0:
                        nc.sync.dma_start(out=dbg_at[hp, :, :], in_=at[:])
                    # ship this pair's at rows toward the pair AllGather
                    if qc < NQC - 1:
                        nc.sync.dma_start(
                            out=ag_in[qc][hp * P:(hp + 1) * P, :], in_=at[:])
                        if hp == 3:
                            nc.gpsimd.collective_compute(
                                "AllGather", ALU.bypass, replica_groups=RG,
                                ins=[ag_in[qc][:]], outs=[ag_out[qc][:]])
                    elif hp < 2:
                        nc.sync.dma_start(
                            out=ag_in3A[hp * P:(hp + 1) * P, :], in_=at[:])
                        if hp == 1:
                            nc.gpsimd.collective_compute(
                                "AllGather", ALU.bypass, replica_groups=RG,
                                ins=[ag_in3A[:]], outs=[ag_out3A[:]])
                    elif hp == 2:
                        nc.sync.dma_start(out=ag_in3B[:], in_=at[:])
                        nc.gpsimd.collective_compute(
                            "AllGather", ALU.bypass, replica_groups=RG,
                            ins=[ag_in3B[:]], outs=[ag_out3B[:]])
                    else:
                        nc.sync.dma_start(out=ag_in3C[:], in_=at[:])
                        nc.gpsimd.collective_compute(
                            "AllGather", ALU.bypass, replica_groups=RG,
                            ins=[ag_in3C[:]], outs=[ag_out3C[:]])

            # ---- main schedule ----
            # chunk 0 QKV up front; chunk qc+1's QKV rides as PE filler in
            # attention(qc); proj(qc) rides in attention(qc+2) (its pair
            # AllGather completed during attention(qc+1)); proj(2) rides in
            # the second half of attention(3); proj(3) is the tail.
            for u in qkv_units(0, xtr=xtr0, pool=ps1, ptag="pa"):
                u()
            if debug:
                nc.sync.dma_start(out=dbg_qt[:], in_=qt_tiles[0][:])
            # chunk-1 QKV also runs up front: its DMAs stream behind chunk
            # 0's on both rings while the PE chews through chunk-0 units, so
            # the start is PE-dense.  Attention chunks then host qkv(qc+2)
            # and the projections concentrate in attention(3), whose causal
            # k-range leaves the most ACT-paced slack to fill.  The upfront
            # units cycle the 3-deep ps1 pool so consecutive units pipeline
            # past their DVE evictions instead of serializing on one bank.
            for u in qkv_units(1, pool=ps1, ptag="pa"):
                u()
            held = []
            for qc in range(NQC):
                if qc + 2 < NQC:
                    units = list(qkv_units(qc + 2))
                    if qc == 1:
                        fillers.extend(units[:6])
                        held = units[6:]
                    else:
                        fillers.extend(units)
                if qc == 2:
                    fillers.extend(held)
                    held = []
                    fillers.extend(proj_units(0))
                if qc == NQC - 1:
                    fillers.extend(proj_units(1))
                    fillers_late.extend(proj_units(2))
                    fillers_late.extend(proj3A_units())
                emit_attention(
                    qc,
                    stage_cb=((lambda q=qc: stage_proj(q - 1)) if qc >= 1
                              else None),
                    late_cb=(stage3_A if qc == NQC - 1 else None))
                drain_fillers()
            stage3_B()
            for u in proj3B_units():
                u()
            stage3_C()
            for u in proj3C_units():
                u()
            if debug:
                nc.sync.dma_start(out=dbg_vpad[:], in_=vpad[:])
                nc.sync.dma_start(out=dbg_kall[:], in_=k_all[:])

    nc.finalize()
    return nc


def _get_nc():
    if "nc" not in _CACHE:
        _CACHE["nc"] = _build()
    return _CACHE["nc"]


def _make_mask():
    # M[p, c] = 1.0 iff (c - 384) >= p; pattern pat slice = cols [384-128*pat:][:CW]
    c = np.arange(MW)[None, :]
    p = np.arange(P)[:, None]
    return ((c - 384) >= p).astype(ml_dtypes.bfloat16)


def make_in_maps(x, W_attn, b_attn, W_proj, b_proj):
    x = np.asarray(x, np.float32)
    W_attn = np.asarray(W_attn, np.float32)
    b_attn = np.asarray(b_attn, np.float32)
    W_proj = np.asarray(W_proj, np.float32)
    b_proj = np.asarray(b_proj, np.float32)
    mask = _make_mask()
    in_maps = []
    for c in range(8):
        b, g = c // 2, c % 2
        sl = slice(g * DL, (g + 1) * DL)
        wqkv_c = np.concatenate([W_attn[:, g * DL:(g + 1) * DL],
                                 W_attn[:, D + g * DL:D + (g + 1) * DL],
                                 W_attn[:, 2 * D + g * DL:2 * D + (g + 1) * DL]],
                                axis=1)
        bqk_c = np.concatenate([b_attn[g * DL:(g + 1) * DL],
                                b_attn[D + g * DL:D + (g + 1) * DL]])
        in_maps.append({
            "xT": np.ascontiguousarray(x[b].T).astype(ml_dtypes.bfloat16),
            "wqkv": np.ascontiguousarray(wqkv_c).astype(ml_dtypes.bfloat16),
            "bqk": np.ascontiguousarray(bqk_c.reshape(8, P).T),
            "bv": b_attn[2 * D + g * DL:2 * D + (g + 1) * DL].reshape(1, DL).copy(),
            "wp": np.ascontiguousarray(W_proj[:, sl]).astype(ml_dtypes.bfloat16),
            "bp": np.ascontiguousarray(b_proj[sl].reshape(4, P).T),
            "maskc": mask,
        })
    return in_maps


def assemble(results):
    out = np.empty((B, S, D), np.float32)
    for c in range(8):
        b, g = c // 2, c % 2
        out[b][:, g * DL:(g + 1) * DL] = results[c]["out"].T
    return out


def kernel(x, W_attn, b_attn, W_proj, b_proj):
    from concourse.bass_utils import run_bass_kernel_spmd
    nc = _get_nc()
    in_maps = make_in_maps(x, W_attn, b_attn, W_proj, b_proj)
    res = run_bass_kernel_spmd(nc, in_maps, core_ids=list(range(8)))
    return assemble(res.results)



# revision 26
# speedup vs baseline: 1.2361x; 1.2033x over previous
"""Distributed causal multi-head attention (GPT-2 style block) for one TRN2 chip.

Sharding over 8 NeuronCores: core c -> (batch b = c//2, head-group g = c%2).
Each core computes QKV for its batch restricted to its 8 heads (tensor-
parallel column split of W_attn) and runs causal attention for those heads.
The pair of cores sharing a batch AllGathers the per-head attention output
(bf16, split in two halves per q-chunk so the collective overlaps attention),
then each core contracts the FULL 1024 head features against its own 512
columns of W_proj, producing a disjoint f32 slice of the output — no
reduce, no post-collective conversion step.

Attention inner loop processes HEAD PAIRS: the even head's K tile lives on
SBUF partitions 0-63 and the odd head's on 64-127, so their score matmuls
(contraction K=64 each) occupy disjoint PE row-groups and execute
concurrently (auto tile_position from base_partition).  One exp ACTIVATE
[128,1024] covers both heads' scores for a k-tile.  Softmax normalization
broadcasts 1/den across partitions with a tiny K=1 PE matmul into the
unused upper half of the PV PSUM bank (no GpSimd in the loop; GpSimd runs
only the collectives + their staging DMAs).

QKV matmul units for the NEXT q-chunk and output-projection units for
chunk qc-2 are injected between attention k-tiles as PE filler work, so the
PE never idles long enough for the HAM clock-gate to re-throttle.  Startup
interleaves the chunk-0 x loads with the V-weight columns on the SP HWDGE
ring while the Q/K weight columns and constants stream on the ACT ring.

Matmul dtypes: f32r (full-rate fp32 at N=512, ~1e-4 rel err) for QKV +
scores, bf16 for exp(P)/V and the output projection.  Softmax runs without
max-subtraction (logits are bounded), with the denominator computed by
augmenting V with a ones column so P@[V|1] yields numerator + denominator.
"""
import numpy as np
import ml_dtypes

B, S, D = 4, 2048, 1024
H, HD, HPC = 16, 64, 8
DL = HPC * HD            # 512 local head features / local out columns
P = 128
CW = 512                 # q-chunk width
NQC = S // CW            # 4
NKT = S // P             # 16
KC = D // P              # 8 contraction chunks of 128
KC2 = 4                  # fp8 DoubleRow super-chunks of 256 (2 per partition)
VW = 65                  # per-head V width incl. ones column
MW = 384 + CW            # compacted causal mask width

_CACHE: dict = {}


def _build(debug=False):
    from concourse import bacc
    import concourse.mybir as mybir
    from concourse.tile import TileContext, add_dep_helper

    F32, F32R, BF16 = mybir.dt.float32, mybir.dt.float32r, mybir.dt.bfloat16
    AF = mybir.ActivationFunctionType
    ALU = mybir.AluOpType

    nc = bacc.Bacc(trn_type="TRN2", num_devices=8)
    if debug:
        dbg_vpad = nc.declare_dram_parameter(
            "dbg_vpad", [P, NKT, HPC * VW], BF16, isOutput=True)
        dbg_rc = nc.declare_dram_parameter(
            "dbg_rc", [NQC * HPC, CW], F32, isOutput=True)
        dbg_at = nc.declare_dram_parameter(
            "dbg_at", [4, P, CW], BF16, isOutput=True)
        dbg_agt = nc.declare_dram_parameter(
            "dbg_agt", [P, KC, CW], BF16, isOutput=True)
        dbg_qt = nc.declare_dram_parameter(
            "dbg_qt", [P, 4, CW], BF16, isOutput=True)
        dbg_kall = nc.declare_dram_parameter(
            "dbg_kall", [P, 4, S], BF16, isOutput=True)
    F8 = mybir.dt.float8e4
    # QKV operands arrive fp8, DoubleRow-interleaved on the host:
    # [partition p, super-chunk c, slot s, col] with contraction dim
    # d = c*256 + s*128 + p.  Weights are pre-scaled by 64 (fp8 range);
    # the 64x comes out in the exp scale and the denominator column.
    xT8 = nc.declare_dram_parameter("xT8", [P, KC2, 2, S], F8, isOutput=False)
    xTb = nc.declare_dram_parameter("xTb", [D, S], BF16, isOutput=False)
    wqkv8 = nc.declare_dram_parameter(
        "wqkv8", [P, KC2, 2, 2 * DL], F8, isOutput=False)
    wv = nc.declare_dram_parameter("wv", [D, DL], BF16, isOutput=False)
    bqk = nc.declare_dram_parameter("bqk", [P, 8], F32, isOutput=False)
    bv = nc.declare_dram_parameter("bv", [1, DL], F32, isOutput=False)
    wp = nc.declare_dram_parameter("wp", [D, DL], BF16, isOutput=False)
    bp = nc.declare_dram_parameter("bp", [P, 4], F32, isOutput=False)
    maskc = nc.declare_dram_parameter("maskc", [P, MW], BF16, isOutput=False)
    out_ext = nc.declare_dram_parameter("out", [DL, S], F32, isOutput=True)

    # per-chunk collective buffers; separate tensors avoid whole-tensor WAR
    # serialization across chunks.  Chunks 0-2 gather all 4 at tiles in one
    # AG; the last chunk splits in half so the first half overlaps attention.
    ag_in = [nc.dram_tensor(f"ag_in_{qc}", [4 * P, CW], BF16)
             for qc in range(NQC - 1)]
    ag_out = [nc.dram_tensor(f"ag_out_{qc}", [8 * P, CW], BF16)
              for qc in range(NQC - 1)]
    # last chunk: three collectives of decreasing size so the final exposed
    # one carries only a single head-pair (128 KB, ~6us latency).
    ag_in3A = nc.dram_tensor("ag_in_3A", [2 * P, CW], BF16)
    ag_out3A = nc.dram_tensor("ag_out_3A", [4 * P, CW], BF16)
    ag_in3B = nc.dram_tensor("ag_in_3B", [P, CW], BF16)
    ag_out3B = nc.dram_tensor("ag_out_3B", [2 * P, CW], BF16)
    ag_in3C = nc.dram_tensor("ag_in_3C", [P, CW], BF16)
    ag_out3C = nc.dram_tensor("ag_out_3C", [2 * P, CW], BF16)
    RG = [[0, 1], [2, 3], [4, 5], [6, 7]]

    with TileContext(nc) as tc:
        with tc.tile_pool(name="const", bufs=1) as constp, \
             tc.tile_pool(name="persist", bufs=1) as perp, \
             tc.tile_pool(name="wq", bufs=1) as wqp, \
             tc.tile_pool(name="xt", bufs=2) as xtp, \
             tc.tile_pool(name="qtp", bufs=3) as qtp, \
             tc.tile_pool(name="wpp", bufs=1) as wpp, \
             tc.tile_pool(name="ptp", bufs=6) as ptp, \
             tc.tile_pool(name="atp", bufs=2) as atp, \
             tc.tile_pool(name="smallp", bufs=1) as smallp, \
             tc.tile_pool(name="agtp", bufs=2) as agtp, \
             tc.tile_pool(name="otp", bufs=2) as otp, \
             tc.tile_pool(name="otap", bufs=8) as otap, \
             tc.tile_pool(name="ps3", bufs=2, space="PSUM") as ps3, \
             tc.tile_pool(name="ps1", bufs=3, space="PSUM") as ps1, \
             tc.tile_pool(name="psq", bufs=1, space="PSUM") as psq:

            # ---- chunk-0 x alone on the SP ring; V-weight columns head the
            # ACT ring, so both streams land kc-by-kc in parallel and the
            # first v_unit matmuls unblock at ~3us ----
            # distinct tags: same-size untagged tiles in one pool share a
            # buffer ring, which would alias these two weight tiles.
            wq_t = wqp.tile([P, KC2, 2, 2 * DL], F8, tag="wq8")
            wv_t = wqp.tile([P, KC, DL], BF16, tag="wv")
            xtr0 = xtp.tile([P, KC2, 2, CW], F8, tag="xtr", name="xtr_0")
            xtrv0 = xtp.tile([P, KC, CW], BF16, tag="xtrv", name="xtrv_0")
            # The GpSimd FIFO must stay in program order: a staging DMA that
            # waits on an AllGather, scheduled ahead of a softmax broadcast,
            # head-of-line-blocks the whole attention pipeline.  Chain every
            # gpsimd-engine instruction to the previous one (collective
            # triggers live on the separate Collectives proc and are NOT
            # chained — waiting on them would serialize chunks).
            _gp_prev = [None]

            def gp(inst):
                if _gp_prev[0] is not None:
                    add_dep_helper(inst.ins, _gp_prev[0].ins,
                                   reason="gpsimd program order")
                _gp_prev[0] = inst
                return inst

            bv_stage = constp.tile([1, DL], F32)
            nc.scalar.dma_start(out=bv_stage[:], in_=bv[:])
            for kc in range(KC):
                nc.sync.dma_start(out=xtrv0[:, kc, :],
                                  in_=xTb[kc * P:(kc + 1) * P, 0:CW])
                nc.scalar.dma_start(out=wv_t[:, kc, :],
                                    in_=wv[kc * P:(kc + 1) * P, :])
            for c in range(KC2):
                nc.sync.dma_start(out=xtr0[:, c, :, :],
                                  in_=xT8[:, c, :, 0:CW])

            # q-weight columns follow x on the SP ring; k-weight columns,
            # constants and wp follow the v columns on the ACT ring.
            for c in range(KC2):
                nc.sync.dma_start(out=wq_t[:, c, :, 0:DL],
                                  in_=wqkv8[:, c, :, 0:DL])
            bqk_t = constp.tile([P, 8], F32)
            nc.scalar.dma_start(out=bqk_t[:], in_=bqk[:])
            bp_t = constp.tile([P, 4], F32)
            nc.scalar.dma_start(out=bp_t[:], in_=bp[:])
            maskr = constp.tile([P, MW], BF16)
            nc.scalar.dma_start(out=maskr[:], in_=maskc[:])
            # the 128x128 causal triangle, duplicated so one tensor_mul
            # masks both heads' diagonal strips via a [P, 2, 128] AP.
            maskd = constp.tile([P, 2, P], BF16)
            nc.vector.tensor_copy(out=maskd[:, 0, :],
                                  in_=maskr[:, 384:384 + P])
            nc.vector.tensor_copy(out=maskd[:, 1, :],
                                  in_=maskr[:, 384:384 + P])
            bias_bc = constp.tile([P, DL], F32)
            gp(nc.gpsimd.partition_broadcast(bias_bc[:], bv_stage[:]))

            # ---- long-lived activations ----
            k_all = perp.tile([P, 4, S], BF16)
            vpad = perp.tile([P, NKT, HPC * VW], BF16)   # v + ones col per head
            ones_sb = constp.tile([P, NKT * HPC], BF16)
            nc.vector.memset(ones_sb[:], 1.0)
            nc.vector.tensor_copy(
                out=vpad[:].rearrange("p nk (h c) -> p (nk h) c",
                                      c=VW)[:, :, HD:VW],
                in_=ones_sb[:].unsqueeze(2))

            for c in range(KC2):
                nc.scalar.dma_start(out=wq_t[:, c, :, DL:2 * DL],
                                    in_=wqkv8[:, c, :, DL:2 * DL])
            wp_t = wpp.tile([P, KC, DL], BF16)
            for kc in range(KC):
                nc.scalar.dma_start(out=wp_t[:, kc, :],
                                    in_=wp[kc * P:(kc + 1) * P, :])

            qt_tiles = {}
            at_sets = {}

            def qkv_units(qc, xtr=None, xtrv=None, pool=None, ptag="psq"):
                """One generator item = one PE unit (8 matmuls + eviction).
                The upfront chunks use the (then idle) 3-deep ps1 pool so
                consecutive units pipeline past their evictions."""
                if pool is None:
                    pool = psq
                if xtr is None:
                    xtr = xtp.tile([P, KC2, 2, CW], F8, tag="xtr",
                                   name=f"xtr_{qc}")
                    xtrv = xtp.tile([P, KC, CW], BF16, tag="xtrv",
                                    name=f"xtrv_{qc}")
                    for kc in range(KC):
                        nc.sync.dma_start(
                            out=xtrv[:, kc, :],
                            in_=xTb[kc * P:(kc + 1) * P,
                                    qc * CW:(qc + 1) * CW])
                    for c in range(KC2):
                        nc.sync.dma_start(
                            out=xtr[:, c, :, :],
                            in_=xT8[:, c, :, qc * CW:(qc + 1) * CW])
                qt = qtp.tile([P, 4, CW], BF16, tag="qt", name=f"qt_{qc}")
                qt_tiles[qc] = qt

                def v_unit(stl):
                    pt = pool.tile([P, CW], F32, tag=ptag, name=f"v_{qc}_{stl}")
                    for kc in range(KC):
                        nc.tensor.matmul(
                            out=pt[:],
                            lhsT=xtrv[:, kc, stl * P:(stl + 1) * P],
                            rhs=wv_t[:, kc, :],
                            start=(kc == 0), stop=(kc == KC - 1))
                    st = qc * 4 + stl
                    nc.vector.tensor_tensor(
                        out=vpad[:, st, :].rearrange(
                            "p (h c) -> p h c", c=VW)[:, :, 0:HD],
                        in0=pt[:].rearrange("p (h c) -> p h c", c=HD),
                        in1=bias_bc[:].rearrange("p (h c) -> p h c", c=HD),
                        op=ALU.add)

                def qk_unit(m):
                    pt = pool.tile([P, CW], F32, tag=ptag, name=f"qk_{qc}_{m}")
                    for c in range(KC2):
                        nc.tensor.matmul(
                            out=pt[:],
                            lhsT=wq_t[:, c, :, m * P:(m + 1) * P],
                            rhs=xtr[:, c, :, :],
                            start=(c == 0), stop=(c == KC2 - 1),
                            perf_mode=mybir.MatmulPerfMode.DoubleRow)
                    dst = (qt[:, m, :] if m < 4
                           else k_all[:, m - 4, qc * CW:(qc + 1) * CW])
                    nc.vector.tensor_scalar_add(
                        out=dst, in0=pt[:], scalar1=bqk_t[:, m:m + 1])

                for stl in range(4):
                    yield lambda stl=stl: v_unit(stl)
                for m in range(8):
                    yield lambda m=m: qk_unit(m)

            agt_tiles = {}

            def stage_proj(qc):
                """Pull the pair-AllGathered at tiles back into SBUF.  On the
                gpsimd queue, scheduled mid-next-chunk so the AG has landed."""
                agt = agtp.tile([P, KC, CW], BF16, tag="agt", name=f"agt_{qc}")
                agt_tiles[qc] = agt
                if qc < NQC - 1:
                    gp(nc.gpsimd.dma_start(
                        out=agt[:, 0:4, :],
                        in_=ag_out[qc][0:4 * P, :].rearrange(
                            "(k p) s -> p k s", p=P)))
                    gp(nc.gpsimd.dma_start(
                        out=agt[:, 4:8, :],
                        in_=ag_out[qc][4 * P:8 * P, :].rearrange(
                            "(k p) s -> p k s", p=P)))
                if debug and qc == 0:
                    nc.sync.dma_start(out=dbg_agt[:], in_=agt[:])

            def proj_units(qc):
                """Full-feature projection onto this core's 512 out columns:
                4 oc-units of 8 matmuls each, from the AllGathered at tiles."""

                def unit(oc):
                    agt = agt_tiles[qc]
                    pp = psq.tile([P, CW], F32, tag="psq", name=f"pp_{qc}_{oc}")
                    for kc in range(KC):
                        nc.tensor.matmul(
                            out=pp[:],
                            lhsT=wp_t[:, kc, oc * P:(oc + 1) * P],
                            rhs=agt[:, kc, :],
                            start=(kc == 0), stop=(kc == KC - 1))
                    ot = otp.tile([P, CW], F32, tag="ot", name=f"ot_{qc}_{oc}")
                    nc.vector.tensor_scalar_add(out=ot[:], in0=pp[:],
                                                scalar1=bp_t[:, oc:oc + 1])
                    nc.sync.dma_start(
                        out=out_ext[oc * P:(oc + 1) * P,
                                    qc * CW:(qc + 1) * CW],
                        in_=ot[:])

                for oc in range(4):
                    yield lambda oc=oc: unit(oc)

            # ---- last-chunk projection, staged per collective: A (hp0+hp1,
            # gathered mid-chunk) runs as attention filler; B (hp2) right
            # after attention while the final C (hp3) collective flies; C
            # trails with just 8 matmuls and ring-parallel output DMAs ----
            agt3 = {}
            ota_tiles = {}
            ota2_tiles = {}

            def stage3_A():
                agt = agtp.tile([P, KC, CW], BF16, tag="agt", name="agt_3")
                agt3["t"] = agt
                gp(nc.gpsimd.dma_start(
                    out=agt[:, 0:2, :],
                    in_=ag_out3A[0:2 * P, :].rearrange("(k p) s -> p k s",
                                                       p=P)))
                gp(nc.gpsimd.dma_start(
                    out=agt[:, 4:6, :],
                    in_=ag_out3A[2 * P:4 * P, :].rearrange(
                        "(k p) s -> p k s", p=P)))

            def stage3_B():
                # tail-only: the sync ring is idle here and HWDGE has lower
                # first-byte latency than the SWDGE path.
                agt = agt3["t"]
                nc.sync.dma_start(out=agt[:, 2, :], in_=ag_out3B[0:P, :])
                nc.sync.dma_start(out=agt[:, 6, :], in_=ag_out3B[P:2 * P, :])

            def stage3_C():
                agt = agt3["t"]
                nc.sync.dma_start(out=agt[:, 3, :], in_=ag_out3C[0:P, :])
                nc.sync.dma_start(out=agt[:, 7, :], in_=ag_out3C[P:2 * P, :])

            def proj3A_units():
                def unitA(oc):
                    agt = agt3["t"]
                    pp = psq.tile([P, CW], F32, tag="psq", name=f"ppA_{oc}")
                    for j, kc in enumerate((0, 1, 4, 5)):
                        nc.tensor.matmul(
                            out=pp[:],
                            lhsT=wp_t[:, kc, oc * P:(oc + 1) * P],
                            rhs=agt[:, kc, :],
                            start=(j == 0), stop=(j == 3))
                    ota = otap.tile([P, CW], BF16, tag="ota", name=f"ota_{oc}")
                    ota_tiles[oc] = ota
                    nc.vector.tensor_scalar_add(out=ota[:], in0=pp[:],
                                                scalar1=bp_t[:, oc:oc + 1])

                for oc in range(4):
                    yield lambda oc=oc: unitA(oc)

            def proj3B_units():
                def unitB(oc):
                    agt = agt3["t"]
                    # ps1's pa slots are free at the tail; 3-deep buffering
                    # lets the B matmul groups pipeline past their evictions.
                    pp = ps1.tile([P, CW], F32, tag="pa", name=f"ppB_{oc}")
                    for j, kc in enumerate((2, 6)):
                        nc.tensor.matmul(
                            out=pp[:],
                            lhsT=wp_t[:, kc, oc * P:(oc + 1) * P],
                            rhs=agt[:, kc, :],
                            start=(j == 0), stop=(j == 1))
                    ota2 = otap.tile([P, CW], BF16, tag="ota2",
                                     name=f"ota2_{oc}")
                    ota2_tiles[oc] = ota2
                    nc.vector.tensor_tensor(out=ota2[:], in0=pp[:],
                                            in1=ota_tiles[oc][:], op=ALU.add)

                for oc in range(4):
                    yield lambda oc=oc: unitB(oc)

            def proj3C_units():
                def unitC(oc):
                    agt = agt3["t"]
                    pp = ps1.tile([P, CW], F32, tag="pa", name=f"ppC_{oc}")
                    for j, kc in enumerate((3, 7)):
                        nc.tensor.matmul(
                            out=pp[:],
                            lhsT=wp_t[:, kc, oc * P:(oc + 1) * P],
                            rhs=agt[:, kc, :],
                            start=(j == 0), stop=(j == 1))
                    ot = otp.tile([P, CW], F32, tag="ot", name=f"otC_{oc}")
                    nc.vector.tensor_tensor(out=ot[:], in0=pp[:],
                                            in1=ota2_tiles[oc][:], op=ALU.add)
                    dst = out_ext[oc * P:(oc + 1) * P,
                                  (NQC - 1) * CW:NQC * CW]
                    # spread the four final stores across engine rings so the
                    # kernel tail is one DMA deep, not four.
                    if oc == 0:
                        nc.sync.dma_start(out=dst, in_=ot[:])
                    elif oc == 1:
                        nc.scalar.dma_start(out=dst, in_=ot[:])
                    elif oc == 2:
                        nc.scalar.dma_start(out=dst, in_=ot[:])
                    else:
                        nc.sync.dma_start(out=dst, in_=ot[:])

                for oc in range(4):
                    yield lambda oc=oc: unitC(oc)

            fillers = []
            fillers_late = []

            def pop_filler(late_ok):
                # keep 2 units in reserve: drain_fillers emits them after the
                # chunk's attention so the PE stays busy (and HAM warm) across
                # the chunk boundary while the last pair's softmax normalize
                # chain releases its PSUM slots.
                if len(fillers) > 2:
                    fillers.pop(0)()
                elif late_ok and fillers_late:
                    fillers_late.pop(0)()

            def drain_fillers():
                while fillers:
                    fillers.pop(0)()
                while fillers_late:
                    fillers_late.pop(0)()

            def emit_attention(qc, stage_cb=None, late_cb=None):
                qt = qt_tiles[qc]
                at_tiles = [None] * 4
                at_sets[qc] = at_tiles
                kmax = 4 * (qc + 1)

                for hp in range(4):
                    if hp == 2 and stage_cb is not None:
                        stage_cb()
                    if hp == 3 and late_cb is not None:
                        late_cb()
                    h_e, h_o = 2 * hp, 2 * hp + 1
                    qs_e = qt[0:64, hp, :]
                    qs_o = qt[64:128, hp, :]
                    pa_e = ps1.tile([P, CW], F32, tag="pa",
                                    name=f"pa_{qc}_{hp}_e")
                    pa_o = ps1.tile([P, CW], F32, tag="pa",
                                    name=f"pa_{qc}_{hp}_o")
                    # Keep TWO tiles in flight before flushing: the PE queue
                    # is in-order, so PV(k) — which waits on mask(k) (Vector)
                    # — must not sit immediately behind score(k+1).  With
                    # depth 2 the mask latency hides under the next exp.
                    pending = []

                    # Diagonal k-tiles (kt >= 4qc) only have valid queries at
                    # columns >= 128*(kt-4qc): trim the score matmul, the exp
                    # ACTIVATE, the causal-mask multiplies and the PV matmuls
                    # to that column range.  The skipped columns are exactly
                    # the fully-masked region, so this is bit-equivalent.
                    def toff(kt):
                        return max(0, (kt - 4 * qc)) * P

                    def flush(pending, kmax=kmax, pa_e=pa_e, pa_o=pa_o,
                              h_e=h_e, h_o=h_o, qc=qc):
                        kt, ptile = pending
                        off = toff(kt)
                        if kt >= 4 * qc:
                            # only columns [off, off+128) are partially
                            # masked (the 128x128 triangle); columns beyond
                            # are fully valid.  One op covers both heads.
                            p3 = ptile[:].rearrange("p (h w) -> p h w", w=CW)
                            nc.vector.tensor_mul(
                                out=p3[:, :, off:off + P],
                                in0=p3[:, :, off:off + P],
                                in1=maskd[:])
                        nc.tensor.matmul(
                            out=pa_e[0:VW, off:CW],
                            lhsT=vpad[:, kt, h_e * VW:(h_e + 1) * VW],
                            rhs=ptile[:, off:CW],
                            start=(kt == 0), stop=(kt == kmax - 1))
                        nc.tensor.matmul(
                            out=pa_o[0:VW, off:CW],
                            lhsT=vpad[:, kt, h_o * VW:(h_o + 1) * VW],
                            rhs=ptile[:, CW + off:2 * CW],
                            start=(kt == 0), stop=(kt == kmax - 1))

                    for kt in range(kmax):
                        off = toff(kt)
                        pt = ps3.tile([P, 2 * CW], F32, tag="ps3",
                                      name=f"sc_{qc}_{hp}_{kt}")
                        nc.tensor.matmul(
                            out=pt[:, off:CW],
                            lhsT=k_all[0:64, hp, kt * P:(kt + 1) * P],
                            rhs=qt[0:64, hp, off:CW], start=True, stop=True)
                        nc.tensor.matmul(
                            out=pt[:, CW + off:2 * CW],
                            lhsT=k_all[64:128, hp, kt * P:(kt + 1) * P],
                            rhs=qt[64:128, hp, off:CW], start=True, stop=True)
                        if len(pending) >= 2:
                            flush(pending.pop(0))
                        ptile = ptp.tile([P, 2 * CW], BF16, tag="pt",
                                         name=f"pt_{qc}_{hp}_{kt}")
                        if off:
                            nc.scalar.activation(
                                ptile[:].rearrange("p (h w) -> p h w",
                                                   w=CW)[:, :, off:CW],
                                pt[:].rearrange("p (h w) -> p h w",
                                                w=CW)[:, :, off:CW],
                                AF.Exp, scale=0.125 / 4096.0)
                        else:
                            nc.scalar.activation(ptile[:], pt[:], AF.Exp,
                                                 scale=0.125 / 4096.0)
                        pending.append((kt, ptile))
                        pop_filler(late_ok=(hp == 3 and kt >= kmax - 10))
                    while pending:
                        flush(pending.pop(0))

                    # normalize by the ones-row denominator.  Both heads'
                    # denominators bounce into one [1, 1024] tile so the
                    # reciprocal and the partition-broadcast run once per
                    # head pair instead of twice.
                    at = atp.tile([P, CW], BF16, tag=f"at{hp}",
                                  name=f"at_{qc}_{hp}")
                    at_tiles[hp] = at
                    # den must bounce PSUM->SBUF: custom-DVE ops misread
                    # PSUM APs with nonzero base_partition on HW, and PSUM
                    # reads must start 32-aligned.
                    den = smallp.tile([1, 2 * CW], F32, tag="den",
                                      name=f"den_{qc}_{hp}")
                    nc.vector.tensor_copy(out=den[:, 0:CW], in_=pa_e[64:65, :])
                    nc.vector.tensor_copy(out=den[:, CW:2 * CW],
                                          in_=pa_o[64:65, :])
                    rc = smallp.tile([1, 2 * CW], F32, tag="recip",
                                     name=f"rc_{qc}_{hp}")
                    nc.vector.reciprocal_approx_fast(out=rc[:], in_=den[:])
                    bc = smallp.tile([64, 2 * CW], F32, tag="bcast",
                                     name=f"bc_{qc}_{hp}")
                    gp(nc.gpsimd.partition_broadcast(bc[:], rc[:]))
                    nc.vector.tensor_tensor(
                        out=at[0:64, :],
                        in0=pa_e[0:64, :], in1=bc[:, 0:CW], op=ALU.mult)
                    nc.vector.tensor_tensor(
                        out=at[64:128, :],
                        in0=pa_o[0:64, :], in1=bc[:, CW:2 * CW], op=ALU.mult)
                    if debug and qc == 0:
                        nc.sync.dma_start(out=dbg_at[hp, :, :], in_=at[:])
                    # ship this pair's at rows toward the pair AllGather
                    if qc < NQC - 1:
                        nc.sync.dma_start(
                            out=ag_in[qc][hp * P:(hp + 1) * P, :], in_=at[:])
                        if hp == 3:
                            nc.gpsimd.collective_compute(
                                "AllGather", ALU.bypass, replica_groups=RG,
                                ins=[ag_in[qc][:]], outs=[ag_out[qc][:]])
                    elif hp < 2:
                        nc.sync.dma_start(
                            out=ag_in3A[hp * P:(hp + 1) * P, :], in_=at[:])
                        if hp == 1:
                            nc.gpsimd.collective_compute(
                                "AllGather", ALU.bypass, replica_groups=RG,
                                ins=[ag_in3A[:]], outs=[ag_out3A[:]])
                    elif hp == 2:
                        nc.sync.dma_start(out=ag_in3B[:], in_=at[:])
                        nc.gpsimd.collective_compute(
                            "AllGather", ALU.bypass, replica_groups=RG,
                            ins=[ag_in3B[:]], outs=[ag_out3B[:]])
                    else:
                        nc.sync.dma_start(out=ag_in3C[:], in_=at[:])
                        nc.gpsimd.collective_compute(
                            "AllGather", ALU.bypass, replica_groups=RG,
                            ins=[ag_in3C[:]], outs=[ag_out3C[:]])

            # ---- main schedule ----
            # chunk 0 QKV up front; chunk qc+1's QKV rides as PE filler in
            # attention(qc); proj(qc) rides in attention(qc+2) (its pair
            # AllGather completed during attention(qc+1)); proj(2) rides in
            # the second half of attention(3); proj(3) is the tail.
            for u in qkv_units(0, xtr=xtr0, xtrv=xtrv0, pool=ps1, ptag="pa"):
                u()
            if debug:
                nc.sync.dma_start(out=dbg_qt[:], in_=qt_tiles[0][:])
            # chunk-1 QKV also runs up front: its DMAs stream behind chunk
            # 0's on both rings while the PE chews through chunk-0 units, so
            # the start is PE-dense.  Attention chunks then host qkv(qc+2)
            # and the projections concentrate in attention(3), whose causal
            # k-range leaves the most ACT-paced slack to fill.  The upfront
            # units cycle the 3-deep ps1 pool so consecutive units pipeline
            # past their DVE evictions instead of serializing on one bank.
            for u in qkv_units(1, pool=ps1, ptag="pa"):
                u()
            held = []
            for qc in range(NQC):
                if qc + 2 < NQC:
                    units = list(qkv_units(qc + 2))
                    if qc == 1:
                        fillers.extend(units[:6])
                        held = units[6:]
                    else:
                        fillers.extend(units)
                if qc == 2:
                    fillers.extend(held)
                    held = []
                    fillers.extend(proj_units(0))
                if qc == NQC - 1:
                    fillers.extend(proj_units(1))
                    fillers_late.extend(proj_units(2))
                    fillers_late.extend(proj3A_units())
                emit_attention(
                    qc,
                    stage_cb=((lambda q=qc: stage_proj(q - 1)) if qc >= 1
                              else None),
                    late_cb=(stage3_A if qc == NQC - 1 else None))
                drain_fillers()
            stage3_B()
            for u in proj3B_units():
                u()
            stage3_C()
            for u in proj3C_units():
                u()
            if debug:
                nc.sync.dma_start(out=dbg_vpad[:], in_=vpad[:])
                nc.sync.dma_start(out=dbg_kall[:], in_=k_all[:])

    nc.finalize()
    return nc


def _get_nc():
    if "nc" not in _CACHE:
        _CACHE["nc"] = _build()
    return _CACHE["nc"]


def _make_mask():
    # M[p, c] = 1.0 iff (c - 384) >= p; pattern pat slice = cols [384-128*pat:][:CW]
    c = np.arange(MW)[None, :]
    p = np.arange(P)[:, None]
    return ((c - 384) >= p).astype(ml_dtypes.bfloat16)


def _dr_interleave(a):
    """[D, N] -> [P, KC2, 2, N] with contraction dim d = c*256 + s*128 + p,
    matching the device-side DoubleRow access pattern."""
    Dd, N = a.shape
    return np.ascontiguousarray(
        a.reshape(KC2, 2, P, N).transpose(2, 0, 1, 3))


def make_in_maps(x, W_attn, b_attn, W_proj, b_proj):
    x = np.asarray(x, np.float32)
    W_attn = np.asarray(W_attn, np.float32)
    b_attn = np.asarray(b_attn, np.float32)
    W_proj = np.asarray(W_proj, np.float32)
    b_proj = np.asarray(b_proj, np.float32)
    mask = _make_mask()
    F8 = ml_dtypes.float8_e4m3fn
    in_maps = []
    for c in range(8):
        b, g = c // 2, c % 2
        sl = slice(g * DL, (g + 1) * DL)
        wqk_c = np.concatenate([W_attn[:, g * DL:(g + 1) * DL],
                                W_attn[:, D + g * DL:D + (g + 1) * DL]],
                               axis=1)
        bqk_c = np.concatenate([b_attn[g * DL:(g + 1) * DL],
                                b_attn[D + g * DL:D + (g + 1) * DL]])
        # q,k weights carry a 64x scale for fp8 range; q,k come out 64x hot,
        # compensated in the exp scale (1/64^2) and the 64x-prescaled bias.
        # The v path stays bf16 (its error feeds the output directly).
        in_maps.append({
            "xT8": _dr_interleave(x[b].T).astype(F8),
            "xTb": np.ascontiguousarray(x[b].T).astype(ml_dtypes.bfloat16),
            "wqkv8": _dr_interleave(wqk_c * 64.0).astype(F8),
            "wv": np.ascontiguousarray(
                W_attn[:, 2 * D + g * DL:2 * D + (g + 1) * DL]
            ).astype(ml_dtypes.bfloat16),
            "bqk": np.ascontiguousarray((64.0 * bqk_c).reshape(8, P).T),
            "bv": b_attn[2 * D + g * DL:2 * D + (g + 1) * DL].reshape(1, DL).copy(),
            "wp": np.ascontiguousarray(W_proj[:, sl]).astype(ml_dtypes.bfloat16),
            "bp": np.ascontiguousarray(b_proj[sl].reshape(4, P).T),
            "maskc": mask,
        })
    return in_maps


def assemble(results):
    out = np.empty((B, S, D), np.float32)
    for c in range(8):
        b, g = c // 2, c % 2
        out[b][:, g * DL:(g + 1) * DL] = results[c]["out"].T
    return out


def kernel(x, W_attn, b_attn, W_proj, b_proj):
    from concourse.bass_utils import run_bass_kernel_spmd
    nc = _get_nc()
    in_maps = make_in_maps(x, W_attn, b_attn, W_proj, b_proj)
    res = run_bass_kernel_spmd(nc, in_maps, core_ids=list(range(8)))
    return assemble(res.results)



# revision 28
# speedup vs baseline: 1.2767x; 1.0328x over previous
"""Distributed causal multi-head attention (GPT-2 style block) for one TRN2 chip.

Sharding over 8 NeuronCores: core c -> (batch b = c//2, head-group g = c%2).
Each core computes QKV for its batch restricted to its 8 heads (tensor-
parallel column split of W_attn) and runs causal attention for those heads.
The pair of cores sharing a batch AllGathers the per-head attention output
(bf16, split in two halves per q-chunk so the collective overlaps attention),
then each core contracts the FULL 1024 head features against its own 512
columns of W_proj, producing a disjoint f32 slice of the output — no
reduce, no post-collective conversion step.

Attention inner loop processes HEAD PAIRS: the even head's K tile lives on
SBUF partitions 0-63 and the odd head's on 64-127, so their score matmuls
(contraction K=64 each) occupy disjoint PE row-groups and execute
concurrently (auto tile_position from base_partition).  One exp ACTIVATE
[128,1024] covers both heads' scores for a k-tile.  Softmax normalization
broadcasts 1/den across partitions with a tiny K=1 PE matmul into the
unused upper half of the PV PSUM bank (no GpSimd in the loop; GpSimd runs
only the collectives + their staging DMAs).

QKV matmul units for the NEXT q-chunk and output-projection units for
chunk qc-2 are injected between attention k-tiles as PE filler work, so the
PE never idles long enough for the HAM clock-gate to re-throttle.  Startup
interleaves the chunk-0 x loads with the V-weight columns on the SP HWDGE
ring while the Q/K weight columns and constants stream on the ACT ring.

Matmul dtypes: f32r (full-rate fp32 at N=512, ~1e-4 rel err) for QKV +
scores, bf16 for exp(P)/V and the output projection.  Softmax runs without
max-subtraction (logits are bounded), with the denominator computed by
augmenting V with a ones column so P@[V|1] yields numerator + denominator.
"""
import numpy as np
import ml_dtypes

B, S, D = 4, 2048, 1024
H, HD, HPC = 16, 64, 8
DL = HPC * HD            # 512 local head features / local out columns
P = 128
CW = 512                 # q-chunk width
NQC = S // CW            # 4
NKT = S // P             # 16
KC = D // P              # 8 contraction chunks of 128
KC2 = 4                  # fp8 DoubleRow super-chunks of 256 (2 per partition)
VW = 65                  # per-head V width incl. ones column (legacy)
VP = 128                 # padded per-head V block: [den@0, pad, v@64:128]
VO = 64                  # v offset inside the block (PSUM reads from base 64
                         # may span 64 partitions; base 32 may only span 32)
MW = 384 + CW            # compacted causal mask width

_CACHE: dict = {}


def _build(debug=False):
    from concourse import bacc
    import concourse.mybir as mybir
    from concourse.tile import TileContext, add_dep_helper

    F32, F32R, BF16 = mybir.dt.float32, mybir.dt.float32r, mybir.dt.bfloat16
    AF = mybir.ActivationFunctionType
    ALU = mybir.AluOpType

    nc = bacc.Bacc(trn_type="TRN2", num_devices=8)
    if debug:
        dbg_vpad = nc.declare_dram_parameter(
            "dbg_vpad", [P, NKT, HPC * VP], BF16, isOutput=True)
        dbg_rc = nc.declare_dram_parameter(
            "dbg_rc", [NQC * HPC, CW], F32, isOutput=True)
        dbg_at = nc.declare_dram_parameter(
            "dbg_at", [4, P, CW], BF16, isOutput=True)
        dbg_agt = nc.declare_dram_parameter(
            "dbg_agt", [P, KC, CW], BF16, isOutput=True)
        dbg_qt = nc.declare_dram_parameter(
            "dbg_qt", [P, 4, CW], BF16, isOutput=True)
        dbg_kall = nc.declare_dram_parameter(
            "dbg_kall", [P, 4, S], BF16, isOutput=True)
    F8 = mybir.dt.float8e4
    # QKV operands arrive fp8, DoubleRow-interleaved on the host:
    # [partition p, super-chunk c, slot s, col] with contraction dim
    # d = c*256 + s*128 + p.  Weights are pre-scaled by 64 (fp8 range);
    # the 64x comes out in the exp scale and the denominator column.
    xT8 = nc.declare_dram_parameter("xT8", [P, KC2, 2, S], F8, isOutput=False)
    xTb = nc.declare_dram_parameter("xTb", [D, S], BF16, isOutput=False)
    wqkv8 = nc.declare_dram_parameter(
        "wqkv8", [P, KC2, 2, 2 * DL], F8, isOutput=False)
    wv = nc.declare_dram_parameter("wv", [D, DL], BF16, isOutput=False)
    bqk = nc.declare_dram_parameter("bqk", [P, 8], F32, isOutput=False)
    bv = nc.declare_dram_parameter("bv", [1, DL], F32, isOutput=False)
    wp = nc.declare_dram_parameter("wp", [D, DL], BF16, isOutput=False)
    bp = nc.declare_dram_parameter("bp", [P, 4], F32, isOutput=False)
    maskc = nc.declare_dram_parameter("maskc", [P, MW], BF16, isOutput=False)
    out_ext = nc.declare_dram_parameter("out", [DL, S], F32, isOutput=True)

    # per-chunk collective buffers; separate tensors avoid whole-tensor WAR
    # serialization across chunks.  Chunks 0-2 gather all 4 at tiles in one
    # AG; the last chunk splits in half so the first half overlaps attention.
    ag_in = [nc.dram_tensor(f"ag_in_{qc}", [4 * P, CW], BF16)
             for qc in range(NQC - 1)]
    ag_out = [nc.dram_tensor(f"ag_out_{qc}", [8 * P, CW], BF16)
              for qc in range(NQC - 1)]
    # last chunk: three collectives of decreasing size so the final exposed
    # one carries only a single head-pair (128 KB, ~6us latency).
    ag_in3A = nc.dram_tensor("ag_in_3A", [2 * P, CW], BF16)
    ag_out3A = nc.dram_tensor("ag_out_3A", [4 * P, CW], BF16)
    ag_in3B = nc.dram_tensor("ag_in_3B", [P, CW], BF16)
    ag_out3B = nc.dram_tensor("ag_out_3B", [2 * P, CW], BF16)
    ag_in3C = nc.dram_tensor("ag_in_3C", [P, CW], BF16)
    ag_out3C = nc.dram_tensor("ag_out_3C", [2 * P, CW], BF16)
    RG = [[0, 1], [2, 3], [4, 5], [6, 7]]

    with TileContext(nc) as tc:
        with tc.tile_pool(name="const", bufs=1) as constp, \
             tc.tile_pool(name="persist", bufs=1) as perp, \
             tc.tile_pool(name="wq", bufs=1) as wqp, \
             tc.tile_pool(name="xt", bufs=2) as xtp, \
             tc.tile_pool(name="qtp", bufs=3) as qtp, \
             tc.tile_pool(name="wpp", bufs=1) as wpp, \
             tc.tile_pool(name="ptp", bufs=6) as ptp, \
             tc.tile_pool(name="atp", bufs=2) as atp, \
             tc.tile_pool(name="smallp", bufs=1) as smallp, \
             tc.tile_pool(name="agtp", bufs=2) as agtp, \
             tc.tile_pool(name="otp", bufs=2) as otp, \
             tc.tile_pool(name="otap", bufs=8) as otap, \
             tc.tile_pool(name="ps3", bufs=2, space="PSUM") as ps3, \
             tc.tile_pool(name="ps1", bufs=3, space="PSUM") as ps1, \
             tc.tile_pool(name="psq", bufs=1, space="PSUM") as psq:

            # ---- chunk-0 x alone on the SP ring; V-weight columns head the
            # ACT ring, so both streams land kc-by-kc in parallel and the
            # first v_unit matmuls unblock at ~3us ----
            # distinct tags: same-size untagged tiles in one pool share a
            # buffer ring, which would alias these two weight tiles.
            wq_t = wqp.tile([P, KC2, 2, 2 * DL], F8, tag="wq8")
            wv_t = wqp.tile([P, KC, DL], BF16, tag="wv")
            xtr0 = xtp.tile([P, KC2, 2, CW], F8, tag="xtr", name="xtr_0")
            xtrv0 = xtp.tile([P, KC, CW], BF16, tag="xtrv", name="xtrv_0")
            # The GpSimd FIFO must stay in program order: a staging DMA that
            # waits on an AllGather, scheduled ahead of a softmax broadcast,
            # head-of-line-blocks the whole attention pipeline.  Chain every
            # gpsimd-engine instruction to the previous one (collective
            # triggers live on the separate Collectives proc and are NOT
            # chained — waiting on them would serialize chunks).
            _gp_prev = [None]

            def gp(inst):
                if _gp_prev[0] is not None:
                    add_dep_helper(inst.ins, _gp_prev[0].ins,
                                   reason="gpsimd program order")
                _gp_prev[0] = inst
                return inst

            bv_stage = constp.tile([1, DL], F32)
            nc.scalar.dma_start(out=bv_stage[:], in_=bv[:])
            for kc in range(KC):
                nc.sync.dma_start(out=xtrv0[:, kc, :],
                                  in_=xTb[kc * P:(kc + 1) * P, 0:CW])
                nc.scalar.dma_start(out=wv_t[:, kc, :],
                                    in_=wv[kc * P:(kc + 1) * P, :])
            for c in range(KC2):
                nc.sync.dma_start(out=xtr0[:, c, :, :],
                                  in_=xT8[:, c, :, 0:CW])

            # q-weight columns follow x on the SP ring; k-weight columns,
            # constants and wp follow the v columns on the ACT ring.
            for c in range(KC2):
                nc.sync.dma_start(out=wq_t[:, c, :, 0:DL],
                                  in_=wqkv8[:, c, :, 0:DL])
            bqk_t = constp.tile([P, 8], F32)
            nc.scalar.dma_start(out=bqk_t[:], in_=bqk[:])
            bp_t = constp.tile([P, 4], F32)
            nc.scalar.dma_start(out=bp_t[:], in_=bp[:])
            maskr = constp.tile([P, MW], BF16)
            nc.scalar.dma_start(out=maskr[:], in_=maskc[:])
            # the 128x128 causal triangle, duplicated so one tensor_mul
            # masks both heads' diagonal strips via a [P, 2, 128] AP.
            maskd = constp.tile([P, 2, P], BF16)
            nc.vector.tensor_copy(out=maskd[:, 0, :],
                                  in_=maskr[:, 384:384 + P])
            nc.vector.tensor_copy(out=maskd[:, 1, :],
                                  in_=maskr[:, 384:384 + P])
            bias_bc = constp.tile([P, DL], F32)
            gp(nc.gpsimd.partition_broadcast(bias_bc[:], bv_stage[:]))

            # ---- long-lived activations ----
            k_all = perp.tile([P, 4, S], BF16)
            vpad = perp.tile([P, NKT, HPC * VP], BF16)   # v + ones col per head
            ones_sb = constp.tile([P, NKT * HPC], BF16)
            nc.vector.memset(ones_sb[:], 1.0)
            # the denominator (ones) column sits at offset 0 of each
            # per-head block so it lands on PSUM partition 0, where the
            # custom-DVE reciprocal can read it directly; v occupies the
            # 32-aligned [VO, VO+64) range so the at multiply is legal.
            nc.vector.tensor_copy(
                out=vpad[:].rearrange("p nk (h c) -> p (nk h) c",
                                      c=VP)[:, :, 0:1],
                in_=ones_sb[:].unsqueeze(2))

            for c in range(KC2):
                nc.scalar.dma_start(out=wq_t[:, c, :, DL:2 * DL],
                                    in_=wqkv8[:, c, :, DL:2 * DL])
            wp_t = wpp.tile([P, KC, DL], BF16)
            for kc in range(KC):
                nc.scalar.dma_start(out=wp_t[:, kc, :],
                                    in_=wp[kc * P:(kc + 1) * P, :])

            qt_tiles = {}
            at_sets = {}

            def qkv_units(qc, xtr=None, xtrv=None, pool=None, ptag="psq"):
                """One generator item = one PE unit (8 matmuls + eviction).
                The upfront chunks use the (then idle) 3-deep ps1 pool so
                consecutive units pipeline past their evictions."""
                if pool is None:
                    pool = psq
                if xtr is None:
                    xtr = xtp.tile([P, KC2, 2, CW], F8, tag="xtr",
                                   name=f"xtr_{qc}")
                    xtrv = xtp.tile([P, KC, CW], BF16, tag="xtrv",
                                    name=f"xtrv_{qc}")
                    for kc in range(KC):
                        nc.sync.dma_start(
                            out=xtrv[:, kc, :],
                            in_=xTb[kc * P:(kc + 1) * P,
                                    qc * CW:(qc + 1) * CW])
                    for c in range(KC2):
                        nc.sync.dma_start(
                            out=xtr[:, c, :, :],
                            in_=xT8[:, c, :, qc * CW:(qc + 1) * CW])
                qt = qtp.tile([P, 4, CW], BF16, tag="qt", name=f"qt_{qc}")
                qt_tiles[qc] = qt

                def v_unit(stl):
                    pt = pool.tile([P, CW], F32, tag=ptag, name=f"v_{qc}_{stl}")
                    for kc in range(KC):
                        nc.tensor.matmul(
                            out=pt[:],
                            lhsT=xtrv[:, kc, stl * P:(stl + 1) * P],
                            rhs=wv_t[:, kc, :],
                            start=(kc == 0), stop=(kc == KC - 1))
                    st = qc * 4 + stl
                    nc.vector.tensor_tensor(
                        out=vpad[:, st, :].rearrange(
                            "p (h c) -> p h c", c=VP)[:, :, VO:VO + HD],
                        in0=pt[:].rearrange("p (h c) -> p h c", c=HD),
                        in1=bias_bc[:].rearrange("p (h c) -> p h c", c=HD),
                        op=ALU.add)

                def qk_unit(m):
                    pt = pool.tile([P, CW], F32, tag=ptag, name=f"qk_{qc}_{m}")
                    for c in range(KC2):
                        nc.tensor.matmul(
                            out=pt[:],
                            lhsT=wq_t[:, c, :, m * P:(m + 1) * P],
                            rhs=xtr[:, c, :, :],
                            start=(c == 0), stop=(c == KC2 - 1),
                            perf_mode=mybir.MatmulPerfMode.DoubleRow)
                    dst = (qt[:, m, :] if m < 4
                           else k_all[:, m - 4, qc * CW:(qc + 1) * CW])
                    nc.vector.tensor_scalar_add(
                        out=dst, in0=pt[:], scalar1=bqk_t[:, m:m + 1])

                for stl in range(4):
                    yield lambda stl=stl: v_unit(stl)
                for m in range(8):
                    yield lambda m=m: qk_unit(m)

            agt_tiles = {}

            def stage_proj(qc):
                """Pull the pair-AllGathered at tiles back into SBUF.  On the
                gpsimd queue, scheduled mid-next-chunk so the AG has landed."""
                agt = agtp.tile([P, KC, CW], BF16, tag="agt", name=f"agt_{qc}")
                agt_tiles[qc] = agt
                if qc < NQC - 1:
                    gp(nc.gpsimd.dma_start(
                        out=agt[:, 0:4, :],
                        in_=ag_out[qc][0:4 * P, :].rearrange(
                            "(k p) s -> p k s", p=P)))
                    gp(nc.gpsimd.dma_start(
                        out=agt[:, 4:8, :],
                        in_=ag_out[qc][4 * P:8 * P, :].rearrange(
                            "(k p) s -> p k s", p=P)))
                if debug and qc == 0:
                    nc.sync.dma_start(out=dbg_agt[:], in_=agt[:])

            def proj_units(qc):
                """Full-feature projection onto this core's 512 out columns:
                4 oc-units of 8 matmuls each, from the AllGathered at tiles."""

                def unit(oc):
                    agt = agt_tiles[qc]
                    pp = psq.tile([P, CW], F32, tag="psq", name=f"pp_{qc}_{oc}")
                    for kc in range(KC):
                        nc.tensor.matmul(
                            out=pp[:],
                            lhsT=wp_t[:, kc, oc * P:(oc + 1) * P],
                            rhs=agt[:, kc, :],
                            start=(kc == 0), stop=(kc == KC - 1))
                    ot = otp.tile([P, CW], F32, tag="ot", name=f"ot_{qc}_{oc}")
                    nc.vector.tensor_scalar_add(out=ot[:], in0=pp[:],
                                                scalar1=bp_t[:, oc:oc + 1])
                    nc.sync.dma_start(
                        out=out_ext[oc * P:(oc + 1) * P,
                                    qc * CW:(qc + 1) * CW],
                        in_=ot[:])

                for oc in range(4):
                    yield lambda oc=oc: unit(oc)

            # ---- last-chunk projection, staged per collective: A (hp0+hp1,
            # gathered mid-chunk) runs as attention filler; B (hp2) right
            # after attention while the final C (hp3) collective flies; C
            # trails with just 8 matmuls and ring-parallel output DMAs ----
            agt3 = {}
            ota_tiles = {}
            ota2_tiles = {}

            def stage3_A():
                agt = agtp.tile([P, KC, CW], BF16, tag="agt", name="agt_3")
                agt3["t"] = agt
                gp(nc.gpsimd.dma_start(
                    out=agt[:, 0:2, :],
                    in_=ag_out3A[0:2 * P, :].rearrange("(k p) s -> p k s",
                                                       p=P)))
                gp(nc.gpsimd.dma_start(
                    out=agt[:, 4:6, :],
                    in_=ag_out3A[2 * P:4 * P, :].rearrange(
                        "(k p) s -> p k s", p=P)))

            def stage3_B():
                # tail-only: the sync ring is idle here and HWDGE has lower
                # first-byte latency than the SWDGE path.
                agt = agt3["t"]
                nc.sync.dma_start(out=agt[:, 2, :], in_=ag_out3B[0:P, :])
                nc.sync.dma_start(out=agt[:, 6, :], in_=ag_out3B[P:2 * P, :])

            def stage3_C():
                agt = agt3["t"]
                nc.sync.dma_start(out=agt[:, 3, :], in_=ag_out3C[0:P, :])
                nc.sync.dma_start(out=agt[:, 7, :], in_=ag_out3C[P:2 * P, :])

            def proj3A_units():
                def unitA(oc):
                    agt = agt3["t"]
                    pp = psq.tile([P, CW], F32, tag="psq", name=f"ppA_{oc}")
                    for j, kc in enumerate((0, 1, 4, 5)):
                        nc.tensor.matmul(
                            out=pp[:],
                            lhsT=wp_t[:, kc, oc * P:(oc + 1) * P],
                            rhs=agt[:, kc, :],
                            start=(j == 0), stop=(j == 3))
                    ota = otap.tile([P, CW], BF16, tag="ota", name=f"ota_{oc}")
                    ota_tiles[oc] = ota
                    nc.vector.tensor_scalar_add(out=ota[:], in0=pp[:],
                                                scalar1=bp_t[:, oc:oc + 1])

                for oc in range(4):
                    yield lambda oc=oc: unitA(oc)

            def proj3B_units():
                def unitB(oc):
                    agt = agt3["t"]
                    # ps1's pa slots are free at the tail; 3-deep buffering
                    # lets the B matmul groups pipeline past their evictions.
                    pp = ps1.tile([P, CW], F32, tag="pa", name=f"ppB_{oc}")
                    for j, kc in enumerate((2, 6)):
                        nc.tensor.matmul(
                            out=pp[:],
                            lhsT=wp_t[:, kc, oc * P:(oc + 1) * P],
                            rhs=agt[:, kc, :],
                            start=(j == 0), stop=(j == 1))
                    ota2 = otap.tile([P, CW], BF16, tag="ota2",
                                     name=f"ota2_{oc}")
                    ota2_tiles[oc] = ota2
                    nc.vector.tensor_tensor(out=ota2[:], in0=pp[:],
                                            in1=ota_tiles[oc][:], op=ALU.add)

                for oc in range(4):
                    yield lambda oc=oc: unitB(oc)

            def proj3C_units():
                def unitC(oc):
                    agt = agt3["t"]
                    pp = ps1.tile([P, CW], F32, tag="pa", name=f"ppC_{oc}")
                    for j, kc in enumerate((3, 7)):
                        nc.tensor.matmul(
                            out=pp[:],
                            lhsT=wp_t[:, kc, oc * P:(oc + 1) * P],
                            rhs=agt[:, kc, :],
                            start=(j == 0), stop=(j == 1))
                    ot = otp.tile([P, CW], F32, tag="ot", name=f"otC_{oc}")
                    nc.vector.tensor_tensor(out=ot[:], in0=pp[:],
                                            in1=ota2_tiles[oc][:], op=ALU.add)
                    dst = out_ext[oc * P:(oc + 1) * P,
                                  (NQC - 1) * CW:NQC * CW]
                    # spread the four final stores across engine rings so the
                    # kernel tail is one DMA deep, not four.
                    if oc == 0:
                        nc.sync.dma_start(out=dst, in_=ot[:])
                    elif oc == 1:
                        nc.scalar.dma_start(out=dst, in_=ot[:])
                    elif oc == 2:
                        nc.scalar.dma_start(out=dst, in_=ot[:])
                    else:
                        nc.sync.dma_start(out=dst, in_=ot[:])

                for oc in range(4):
                    yield lambda oc=oc: unitC(oc)

            fillers = []
            fillers_late = []

            def pop_filler(late_ok):
                # keep 2 units in reserve: drain_fillers emits them after the
                # chunk's attention so the PE stays busy (and HAM warm) across
                # the chunk boundary while the last pair's softmax normalize
                # chain releases its PSUM slots.
                if len(fillers) > 2:
                    fillers.pop(0)()
                elif late_ok and fillers_late:
                    fillers_late.pop(0)()

            def drain_fillers():
                while fillers:
                    fillers.pop(0)()
                while fillers_late:
                    fillers_late.pop(0)()

            def emit_attention(qc, stage_cb=None, late_cb=None):
                qt = qt_tiles[qc]
                at_tiles = [None] * 4
                at_sets[qc] = at_tiles
                kmax = 4 * (qc + 1)

                for hp in range(4):
                    if hp == 2 and stage_cb is not None:
                        stage_cb()
                    if hp == 3 and late_cb is not None:
                        late_cb()
                    h_e, h_o = 2 * hp, 2 * hp + 1
                    qs_e = qt[0:64, hp, :]
                    qs_o = qt[64:128, hp, :]
                    pa_e = ps1.tile([P, CW], F32, tag="pa",
                                    name=f"pa_{qc}_{hp}_e")
                    pa_o = ps1.tile([P, CW], F32, tag="pa",
                                    name=f"pa_{qc}_{hp}_o")
                    # Keep TWO tiles in flight before flushing: the PE queue
                    # is in-order, so PV(k) — which waits on mask(k) (Vector)
                    # — must not sit immediately behind score(k+1).  With
                    # depth 2 the mask latency hides under the next exp.
                    pending = []

                    # Diagonal k-tiles (kt >= 4qc) only have valid queries at
                    # columns >= 128*(kt-4qc): trim the score matmul, the exp
                    # ACTIVATE, the causal-mask multiplies and the PV matmuls
                    # to that column range.  The skipped columns are exactly
                    # the fully-masked region, so this is bit-equivalent.
                    def toff(kt):
                        return max(0, (kt - 4 * qc)) * P

                    def flush(pending, kmax=kmax, pa_e=pa_e, pa_o=pa_o,
                              h_e=h_e, h_o=h_o, qc=qc):
                        kt, ptile = pending
                        off = toff(kt)
                        if kt >= 4 * qc:
                            # only columns [off, off+128) are partially
                            # masked (the 128x128 triangle); columns beyond
                            # are fully valid.  One op covers both heads.
                            p3 = ptile[:].rearrange("p (h w) -> p h w", w=CW)
                            nc.vector.tensor_mul(
                                out=p3[:, :, off:off + P],
                                in0=p3[:, :, off:off + P],
                                in1=maskd[:])
                        nc.tensor.matmul(
                            out=pa_e[0:VP, off:CW],
                            lhsT=vpad[:, kt, h_e * VP:(h_e + 1) * VP],
                            rhs=ptile[:, off:CW],
                            start=(kt == 0), stop=(kt == kmax - 1))
                        nc.tensor.matmul(
                            out=pa_o[0:VP, off:CW],
                            lhsT=vpad[:, kt, h_o * VP:(h_o + 1) * VP],
                            rhs=ptile[:, CW + off:2 * CW],
                            start=(kt == 0), stop=(kt == kmax - 1))

                    for kt in range(kmax):
                        off = toff(kt)
                        pt = ps3.tile([P, 2 * CW], F32, tag="ps3",
                                      name=f"sc_{qc}_{hp}_{kt}")
                        nc.tensor.matmul(
                            out=pt[:, off:CW],
                            lhsT=k_all[0:64, hp, kt * P:(kt + 1) * P],
                            rhs=qt[0:64, hp, off:CW], start=True, stop=True)
                        nc.tensor.matmul(
                            out=pt[:, CW + off:2 * CW],
                            lhsT=k_all[64:128, hp, kt * P:(kt + 1) * P],
                            rhs=qt[64:128, hp, off:CW], start=True, stop=True)
                        if len(pending) >= 2:
                            flush(pending.pop(0))
                        ptile = ptp.tile([P, 2 * CW], BF16, tag="pt",
                                         name=f"pt_{qc}_{hp}_{kt}")
                        if off:
                            nc.scalar.activation(
                                ptile[:].rearrange("p (h w) -> p h w",
                                                   w=CW)[:, :, off:CW],
                                pt[:].rearrange("p (h w) -> p h w",
                                                w=CW)[:, :, off:CW],
                                AF.Exp, scale=0.125 / 4096.0)
                        else:
                            nc.scalar.activation(ptile[:], pt[:], AF.Exp,
                                                 scale=0.125 / 4096.0)
                        pending.append((kt, ptile))
                        pop_filler(late_ok=(hp == 3 and kt >= kmax - 10))
                    while pending:
                        flush(pending.pop(0))

                    # normalize by the ones-row denominator.  Both heads'
                    # denominators bounce into one [1, 1024] tile so the
                    # reciprocal and the partition-broadcast run once per
                    # head pair instead of twice.
                    at = atp.tile([P, CW], BF16, tag=f"at{hp}",
                                  name=f"at_{qc}_{hp}")
                    at_tiles[hp] = at
                    # the denominator is on PSUM partition 0 (base-0 PSUM
                    # reads are safe for custom-DVE ops; only nonzero base
                    # partitions misread) so the reciprocal reads it without
                    # a bounce copy.
                    rc = smallp.tile([1, 2 * CW], F32, tag="recip",
                                     name=f"rc_{qc}_{hp}")
                    nc.vector.reciprocal_approx_fast(out=rc[:, 0:CW],
                                                     in_=pa_e[0:1, :])
                    nc.vector.reciprocal_approx_fast(out=rc[:, CW:2 * CW],
                                                     in_=pa_o[0:1, :])
                    bc = smallp.tile([64, 2 * CW], F32, tag="bcast",
                                     name=f"bc_{qc}_{hp}")
                    gp(nc.gpsimd.partition_broadcast(bc[:], rc[:]))
                    nc.vector.tensor_tensor(
                        out=at[0:64, :],
                        in0=pa_e[VO:VO + 64, :], in1=bc[:, 0:CW], op=ALU.mult)
                    nc.vector.tensor_tensor(
                        out=at[64:128, :],
                        in0=pa_o[VO:VO + 64, :], in1=bc[:, CW:2 * CW],
                        op=ALU.mult)
                    if debug and qc == 0:
                        nc.sync.dma_start(out=dbg_at[hp, :, :], in_=at[:])
                    # ship this pair's at rows toward the pair AllGather
                    if qc < NQC - 1:
                        nc.sync.dma_start(
                            out=ag_in[qc][hp * P:(hp + 1) * P, :], in_=at[:])
                        if hp == 3:
                            nc.gpsimd.collective_compute(
                                "AllGather", ALU.bypass, replica_groups=RG,
                                ins=[ag_in[qc][:]], outs=[ag_out[qc][:]])
                    elif hp < 2:
                        nc.sync.dma_start(
                            out=ag_in3A[hp * P:(hp + 1) * P, :], in_=at[:])
                        if hp == 1:
                            nc.gpsimd.collective_compute(
                                "AllGather", ALU.bypass, replica_groups=RG,
                                ins=[ag_in3A[:]], outs=[ag_out3A[:]])
                    elif hp == 2:
                        nc.sync.dma_start(out=ag_in3B[:], in_=at[:])
                        nc.gpsimd.collective_compute(
                            "AllGather", ALU.bypass, replica_groups=RG,
                            ins=[ag_in3B[:]], outs=[ag_out3B[:]])
                    else:
                        nc.sync.dma_start(out=ag_in3C[:], in_=at[:])
                        nc.gpsimd.collective_compute(
                            "AllGather", ALU.bypass, replica_groups=RG,
                            ins=[ag_in3C[:]], outs=[ag_out3C[:]])

            # ---- main schedule ----
            # chunk 0 QKV up front; chunk qc+1's QKV rides as PE filler in
            # attention(qc); proj(qc) rides in attention(qc+2) (its pair
            # AllGather completed during attention(qc+1)); proj(2) rides in
            # the second half of attention(3); proj(3) is the tail.
            for u in qkv_units(0, xtr=xtr0, xtrv=xtrv0, pool=ps1, ptag="pa"):
                u()
            if debug:
                nc.sync.dma_start(out=dbg_qt[:], in_=qt_tiles[0][:])
            # chunk-1 QKV also runs up front: its DMAs stream behind chunk
            # 0's on both rings while the PE chews through chunk-0 units, so
            # the start is PE-dense.  Attention chunks then host qkv(qc+2)
            # and the projections concentrate in attention(3), whose causal
            # k-range leaves the most ACT-paced slack to fill.  The upfront
            # units cycle the 3-deep ps1 pool so consecutive units pipeline
            # past their DVE evictions instead of serializing on one bank.
            for u in qkv_units(1, pool=ps1, ptag="pa"):
                u()
            # chunk-2 QKV also runs upfront (its x/weight DMAs stream behind
            # chunks 0-1 while the PE chews through them), so attention(0) —
            # the most Vector-loaded chunk (all-diagonal masks + per-hp
            # normalize every 4 tiles) — carries no filler at all.
            for u in qkv_units(2, pool=ps1, ptag="pa"):
                u()
            held = []
            for qc in range(NQC):
                if qc == 1:
                    units = list(qkv_units(3))
                    fillers.extend(units[:6])
                    held = units[6:]
                if qc == 2:
                    fillers.extend(held)
                    held = []
                    fillers.extend(proj_units(0))
                if qc == NQC - 1:
                    fillers.extend(proj_units(1))
                    fillers_late.extend(proj_units(2))
                    fillers_late.extend(proj3A_units())
                emit_attention(
                    qc,
                    stage_cb=((lambda q=qc: stage_proj(q - 1)) if qc >= 1
                              else None),
                    late_cb=(stage3_A if qc == NQC - 1 else None))
                drain_fillers()
            stage3_B()
            for u in proj3B_units():
                u()
            stage3_C()
            for u in proj3C_units():
                u()
            if debug:
                nc.sync.dma_start(out=dbg_vpad[:], in_=vpad[:])
                nc.sync.dma_start(out=dbg_kall[:], in_=k_all[:])

    nc.finalize()
    return nc


def _get_nc():
    if "nc" not in _CACHE:
        _CACHE["nc"] = _build()
    return _CACHE["nc"]


def _make_mask():
    # M[p, c] = 1.0 iff (c - 384) >= p; pattern pat slice = cols [384-128*pat:][:CW]
    c = np.arange(MW)[None, :]
    p = np.arange(P)[:, None]
    return ((c - 384) >= p).astype(ml_dtypes.bfloat16)


def _dr_interleave(a):
    """[D, N] -> [P, KC2, 2, N] with contraction dim d = c*256 + s*128 + p,
    matching the device-side DoubleRow access pattern."""
    Dd, N = a.shape
    return np.ascontiguousarray(
        a.reshape(KC2, 2, P, N).transpose(2, 0, 1, 3))


def make_in_maps(x, W_attn, b_attn, W_proj, b_proj):
    x = np.asarray(x, np.float32)
    W_attn = np.asarray(W_attn, np.float32)
    b_attn = np.asarray(b_attn, np.float32)
    W_proj = np.asarray(W_proj, np.float32)
    b_proj = np.asarray(b_proj, np.float32)
    mask = _make_mask()
    F8 = ml_dtypes.float8_e4m3fn
    in_maps = []
    for c in range(8):
        b, g = c // 2, c % 2
        sl = slice(g * DL, (g + 1) * DL)
        wqk_c = np.concatenate([W_attn[:, g * DL:(g + 1) * DL],
                                W_attn[:, D + g * DL:D + (g + 1) * DL]],
                               axis=1)
        bqk_c = np.concatenate([b_attn[g * DL:(g + 1) * DL],
                                b_attn[D + g * DL:D + (g + 1) * DL]])
        # q,k weights carry a 64x scale for fp8 range; q,k come out 64x hot,
        # compensated in the exp scale (1/64^2) and the 64x-prescaled bias.
        # The v path stays bf16 (its error feeds the output directly).
        in_maps.append({
            "xT8": _dr_interleave(x[b].T).astype(F8),
            "xTb": np.ascontiguousarray(x[b].T).astype(ml_dtypes.bfloat16),
            "wqkv8": _dr_interleave(wqk_c * 64.0).astype(F8),
            "wv": np.ascontiguousarray(
                W_attn[:, 2 * D + g * DL:2 * D + (g + 1) * DL]
            ).astype(ml_dtypes.bfloat16),
            "bqk": np.ascontiguousarray((64.0 * bqk_c).reshape(8, P).T),
            "bv": b_attn[2 * D + g * DL:2 * D + (g + 1) * DL].reshape(1, DL).copy(),
            "wp": np.ascontiguousarray(W_proj[:, sl]).astype(ml_dtypes.bfloat16),
            "bp": np.ascontiguousarray(b_proj[sl].reshape(4, P).T),
            "maskc": mask,
        })
    return in_maps


def assemble(results):
    out = np.empty((B, S, D), np.float32)
    for c in range(8):
        b, g = c // 2, c % 2
        out[b][:, g * DL:(g + 1) * DL] = results[c]["out"].T
    return out


def kernel(x, W_attn, b_attn, W_proj, b_proj):
    from concourse.bass_utils import run_bass_kernel_spmd
    nc = _get_nc()
    in_maps = make_in_maps(x, W_attn, b_attn, W_proj, b_proj)
    res = run_bass_kernel_spmd(nc, in_maps, core_ids=list(range(8)))
    return assemble(res.results)



# revision 29
# speedup vs baseline: 1.2822x; 1.0043x over previous
"""Distributed causal multi-head attention (GPT-2 style block) for one TRN2 chip.

Sharding over 8 NeuronCores: core c -> (batch b = c//2, head-group g = c%2).
Each core computes QKV for its batch restricted to its 8 heads (tensor-
parallel column split of W_attn) and runs causal attention for those heads.
The pair of cores sharing a batch AllGathers the per-head attention output
(bf16, split in two halves per q-chunk so the collective overlaps attention),
then each core contracts the FULL 1024 head features against its own 512
columns of W_proj, producing a disjoint f32 slice of the output — no
reduce, no post-collective conversion step.

Attention inner loop processes HEAD PAIRS: the even head's K tile lives on
SBUF partitions 0-63 and the odd head's on 64-127, so their score matmuls
(contraction K=64 each) occupy disjoint PE row-groups and execute
concurrently (auto tile_position from base_partition).  One exp ACTIVATE
[128,1024] covers both heads' scores for a k-tile.  Softmax normalization
broadcasts 1/den across partitions with a tiny K=1 PE matmul into the
unused upper half of the PV PSUM bank (no GpSimd in the loop; GpSimd runs
only the collectives + their staging DMAs).

QKV matmul units for the NEXT q-chunk and output-projection units for
chunk qc-2 are injected between attention k-tiles as PE filler work, so the
PE never idles long enough for the HAM clock-gate to re-throttle.  Startup
interleaves the chunk-0 x loads with the V-weight columns on the SP HWDGE
ring while the Q/K weight columns and constants stream on the ACT ring.

Matmul dtypes: f32r (full-rate fp32 at N=512, ~1e-4 rel err) for QKV +
scores, bf16 for exp(P)/V and the output projection.  Softmax runs without
max-subtraction (logits are bounded), with the denominator computed by
augmenting V with a ones column so P@[V|1] yields numerator + denominator.
"""
import numpy as np
import ml_dtypes

B, S, D = 4, 2048, 1024
H, HD, HPC = 16, 64, 8
DL = HPC * HD            # 512 local head features / local out columns
P = 128
CW = 512                 # q-chunk width
NQC = S // CW            # 4
NKT = S // P             # 16
KC = D // P              # 8 contraction chunks of 128
KC2 = 4                  # fp8 DoubleRow super-chunks of 256 (2 per partition)
VW = 65                  # per-head V width incl. ones column (legacy)
VP = 128                 # padded per-head V block: [den@0, pad, v@64:128]
VO = 64                  # v offset inside the block (PSUM reads from base 64
                         # may span 64 partitions; base 32 may only span 32)
MW = 384 + CW            # compacted causal mask width

_CACHE: dict = {}


def _build(debug=False):
    from concourse import bacc
    import concourse.mybir as mybir
    from concourse.tile import TileContext, add_dep_helper

    F32, F32R, BF16 = mybir.dt.float32, mybir.dt.float32r, mybir.dt.bfloat16
    AF = mybir.ActivationFunctionType
    ALU = mybir.AluOpType

    nc = bacc.Bacc(trn_type="TRN2", num_devices=8)
    if debug:
        dbg_vpad = nc.declare_dram_parameter(
            "dbg_vpad", [P, NKT, HPC * VP], BF16, isOutput=True)
        dbg_rc = nc.declare_dram_parameter(
            "dbg_rc", [NQC * HPC, CW], F32, isOutput=True)
        dbg_at = nc.declare_dram_parameter(
            "dbg_at", [4, P, CW], BF16, isOutput=True)
        dbg_agt = nc.declare_dram_parameter(
            "dbg_agt", [P, KC, CW], BF16, isOutput=True)
        dbg_qt = nc.declare_dram_parameter(
            "dbg_qt", [P, 4, CW], BF16, isOutput=True)
        dbg_kall = nc.declare_dram_parameter(
            "dbg_kall", [P, 4, S], BF16, isOutput=True)
    F8 = mybir.dt.float8e4
    # QKV operands arrive fp8, DoubleRow-interleaved on the host:
    # [partition p, super-chunk c, slot s, col] with contraction dim
    # d = c*256 + s*128 + p.  Weights are pre-scaled by 64 (fp8 range);
    # the 64x comes out in the exp scale and the denominator column.
    xT8 = nc.declare_dram_parameter("xT8", [P, KC2, 2, S], F8, isOutput=False)
    xTb = nc.declare_dram_parameter("xTb", [D, S], BF16, isOutput=False)
    wqkv8 = nc.declare_dram_parameter(
        "wqkv8", [P, KC2, 2, 2 * DL], F8, isOutput=False)
    wv = nc.declare_dram_parameter("wv", [D, DL], BF16, isOutput=False)
    bqk = nc.declare_dram_parameter("bqk", [P, 8], F32, isOutput=False)
    bv = nc.declare_dram_parameter("bv", [1, DL], F32, isOutput=False)
    wp = nc.declare_dram_parameter("wp", [D, DL], BF16, isOutput=False)
    bp = nc.declare_dram_parameter("bp", [P, 4], F32, isOutput=False)
    maskc = nc.declare_dram_parameter("maskc", [P, MW], BF16, isOutput=False)
    out_ext = nc.declare_dram_parameter("out", [DL, S], F32, isOutput=True)

    # per-chunk collective buffers; separate tensors avoid whole-tensor WAR
    # serialization across chunks.  Chunks 0-2 gather all 4 at tiles in one
    # AG; the last chunk splits in half so the first half overlaps attention.
    ag_in = [nc.dram_tensor(f"ag_in_{qc}", [4 * P, CW], BF16)
             for qc in range(NQC - 1)]
    ag_out = [nc.dram_tensor(f"ag_out_{qc}", [8 * P, CW], BF16)
              for qc in range(NQC - 1)]
    # last chunk: three collectives of decreasing size so the final exposed
    # one carries only a single head-pair (128 KB, ~6us latency).
    ag_in3A = nc.dram_tensor("ag_in_3A", [2 * P, CW], BF16)
    ag_out3A = nc.dram_tensor("ag_out_3A", [4 * P, CW], BF16)
    ag_in3B = nc.dram_tensor("ag_in_3B", [P, CW], BF16)
    ag_out3B = nc.dram_tensor("ag_out_3B", [2 * P, CW], BF16)
    ag_in3C = nc.dram_tensor("ag_in_3C", [P, CW], BF16)
    ag_out3C = nc.dram_tensor("ag_out_3C", [2 * P, CW], BF16)
    RG = [[0, 1], [2, 3], [4, 5], [6, 7]]

    with TileContext(nc) as tc:
        with tc.tile_pool(name="const", bufs=1) as constp, \
             tc.tile_pool(name="persist", bufs=1) as perp, \
             tc.tile_pool(name="wq", bufs=1) as wqp, \
             tc.tile_pool(name="xt", bufs=2) as xtp, \
             tc.tile_pool(name="qtp", bufs=3) as qtp, \
             tc.tile_pool(name="wpp", bufs=1) as wpp, \
             tc.tile_pool(name="ptp", bufs=6) as ptp, \
             tc.tile_pool(name="atp", bufs=2) as atp, \
             tc.tile_pool(name="smallp", bufs=1) as smallp, \
             tc.tile_pool(name="agtp", bufs=2) as agtp, \
             tc.tile_pool(name="otp", bufs=2) as otp, \
             tc.tile_pool(name="otap", bufs=8) as otap, \
             tc.tile_pool(name="ps3", bufs=2, space="PSUM") as ps3, \
             tc.tile_pool(name="ps1", bufs=3, space="PSUM") as ps1, \
             tc.tile_pool(name="psq", bufs=1, space="PSUM") as psq:

            # ---- chunk-0 x alone on the SP ring; V-weight columns head the
            # ACT ring, so both streams land kc-by-kc in parallel and the
            # first v_unit matmuls unblock at ~3us ----
            # distinct tags: same-size untagged tiles in one pool share a
            # buffer ring, which would alias these two weight tiles.
            wq_t = wqp.tile([P, KC2, 2, 2 * DL], F8, tag="wq8")
            wv_t = wqp.tile([P, KC, DL], BF16, tag="wv")
            xtr0 = xtp.tile([P, KC2, 2, CW], F8, tag="xtr", name="xtr_0")
            xtrv0 = xtp.tile([P, KC, CW], BF16, tag="xtrv", name="xtrv_0")
            # The GpSimd FIFO must stay in program order: a staging DMA that
            # waits on an AllGather, scheduled ahead of a softmax broadcast,
            # head-of-line-blocks the whole attention pipeline.  Chain every
            # gpsimd-engine instruction to the previous one (collective
            # triggers live on the separate Collectives proc and are NOT
            # chained — waiting on them would serialize chunks).
            _gp_prev = [None]

            def gp(inst):
                if _gp_prev[0] is not None:
                    add_dep_helper(inst.ins, _gp_prev[0].ins,
                                   reason="gpsimd program order")
                _gp_prev[0] = inst
                return inst

            bv_stage = constp.tile([1, DL], F32)
            nc.scalar.dma_start(out=bv_stage[:], in_=bv[:])
            for kc in range(KC):
                nc.sync.dma_start(out=xtrv0[:, kc, :],
                                  in_=xTb[kc * P:(kc + 1) * P, 0:CW])
                nc.scalar.dma_start(out=wv_t[:, kc, :],
                                    in_=wv[kc * P:(kc + 1) * P, :])
            nc.sync.dma_start(out=xtr0[:], in_=xT8[:, :, :, 0:CW])

            # q-weight columns follow x on the SP ring; k-weight columns,
            # constants and wp follow the v columns on the ACT ring.
            nc.sync.dma_start(out=wq_t[:, :, :, 0:DL],
                              in_=wqkv8[:, :, :, 0:DL])
            bqk_t = constp.tile([P, 8], F32)
            nc.scalar.dma_start(out=bqk_t[:], in_=bqk[:])
            bp_t = constp.tile([P, 4], F32)
            nc.scalar.dma_start(out=bp_t[:], in_=bp[:])
            maskr = constp.tile([P, MW], BF16)
            nc.scalar.dma_start(out=maskr[:], in_=maskc[:])
            # the 128x128 causal triangle, duplicated so one tensor_mul
            # masks both heads' diagonal strips via a [P, 2, 128] AP.
            maskd = constp.tile([P, 2, P], BF16)
            nc.vector.tensor_copy(out=maskd[:, 0, :],
                                  in_=maskr[:, 384:384 + P])
            nc.vector.tensor_copy(out=maskd[:, 1, :],
                                  in_=maskr[:, 384:384 + P])
            bias_bc = constp.tile([P, DL], F32)
            gp(nc.gpsimd.partition_broadcast(bias_bc[:], bv_stage[:]))

            # ---- long-lived activations ----
            k_all = perp.tile([P, 4, S], BF16)
            vpad = perp.tile([P, NKT, HPC * VP], BF16)   # v + ones col per head
            ones_sb = constp.tile([P, NKT * HPC], BF16)
            nc.vector.memset(ones_sb[:], 1.0)
            # the denominator (ones) column sits at offset 0 of each
            # per-head block so it lands on PSUM partition 0, where the
            # custom-DVE reciprocal can read it directly; v occupies the
            # 32-aligned [VO, VO+64) range so the at multiply is legal.
            nc.vector.tensor_copy(
                out=vpad[:].rearrange("p nk (h c) -> p (nk h) c",
                                      c=VP)[:, :, 0:1],
                in_=ones_sb[:].unsqueeze(2))

            nc.scalar.dma_start(out=wq_t[:, :, :, DL:2 * DL],
                                in_=wqkv8[:, :, :, DL:2 * DL])
            wp_t = wpp.tile([P, KC, DL], BF16)
            nc.scalar.dma_start(
                out=wp_t[:],
                in_=wp[:].rearrange("(k p) c -> p k c", p=P))

            qt_tiles = {}
            at_sets = {}

            def qkv_units(qc, xtr=None, xtrv=None, pool=None, ptag="psq"):
                """One generator item = one PE unit (8 matmuls + eviction).
                The upfront chunks use the (then idle) 3-deep ps1 pool so
                consecutive units pipeline past their evictions."""
                if pool is None:
                    pool = psq
                if xtr is None:
                    xtr = xtp.tile([P, KC2, 2, CW], F8, tag="xtr",
                                   name=f"xtr_{qc}")
                    xtrv = xtp.tile([P, KC, CW], BF16, tag="xtrv",
                                    name=f"xtrv_{qc}")
                    # single batched transfers: the HWDGE ring costs ~0.6us
                    # of trigger time per DMA, so 12 small loads per chunk
                    # would saturate the ring during the upfront phase.
                    nc.sync.dma_start(
                        out=xtrv[:],
                        in_=xTb[:, qc * CW:(qc + 1) * CW].rearrange(
                            "(k p) s -> p k s", p=P))
                    nc.sync.dma_start(
                        out=xtr[:],
                        in_=xT8[:, :, :, qc * CW:(qc + 1) * CW])
                qt = qtp.tile([P, 4, CW], BF16, tag="qt", name=f"qt_{qc}")
                qt_tiles[qc] = qt

                def v_unit(stl):
                    pt = pool.tile([P, CW], F32, tag=ptag, name=f"v_{qc}_{stl}")
                    for kc in range(KC):
                        nc.tensor.matmul(
                            out=pt[:],
                            lhsT=xtrv[:, kc, stl * P:(stl + 1) * P],
                            rhs=wv_t[:, kc, :],
                            start=(kc == 0), stop=(kc == KC - 1))
                    st = qc * 4 + stl
                    nc.vector.tensor_tensor(
                        out=vpad[:, st, :].rearrange(
                            "p (h c) -> p h c", c=VP)[:, :, VO:VO + HD],
                        in0=pt[:].rearrange("p (h c) -> p h c", c=HD),
                        in1=bias_bc[:].rearrange("p (h c) -> p h c", c=HD),
                        op=ALU.add)

                def qk_unit(m):
                    pt = pool.tile([P, CW], F32, tag=ptag, name=f"qk_{qc}_{m}")
                    for c in range(KC2):
                        nc.tensor.matmul(
                            out=pt[:],
                            lhsT=wq_t[:, c, :, m * P:(m + 1) * P],
                            rhs=xtr[:, c, :, :],
                            start=(c == 0), stop=(c == KC2 - 1),
                            perf_mode=mybir.MatmulPerfMode.DoubleRow)
                    dst = (qt[:, m, :] if m < 4
                           else k_all[:, m - 4, qc * CW:(qc + 1) * CW])
                    nc.vector.tensor_scalar_add(
                        out=dst, in0=pt[:], scalar1=bqk_t[:, m:m + 1])

                for stl in range(4):
                    yield lambda stl=stl: v_unit(stl)
                for m in range(8):
                    yield lambda m=m: qk_unit(m)

            agt_tiles = {}

            def stage_proj(qc):
                """Pull the pair-AllGathered at tiles back into SBUF.  On the
                gpsimd queue, scheduled mid-next-chunk so the AG has landed."""
                agt = agtp.tile([P, KC, CW], BF16, tag="agt", name=f"agt_{qc}")
                agt_tiles[qc] = agt
                if qc < NQC - 1:
                    gp(nc.gpsimd.dma_start(
                        out=agt[:, 0:4, :],
                        in_=ag_out[qc][0:4 * P, :].rearrange(
                            "(k p) s -> p k s", p=P)))
                    gp(nc.gpsimd.dma_start(
                        out=agt[:, 4:8, :],
                        in_=ag_out[qc][4 * P:8 * P, :].rearrange(
                            "(k p) s -> p k s", p=P)))
                if debug and qc == 0:
                    nc.sync.dma_start(out=dbg_agt[:], in_=agt[:])

            def proj_units(qc):
                """Full-feature projection onto this core's 512 out columns:
                4 oc-units of 8 matmuls each, from the AllGathered at tiles."""

                def unit(oc):
                    agt = agt_tiles[qc]
                    pp = psq.tile([P, CW], F32, tag="psq", name=f"pp_{qc}_{oc}")
                    for kc in range(KC):
                        nc.tensor.matmul(
                            out=pp[:],
                            lhsT=wp_t[:, kc, oc * P:(oc + 1) * P],
                            rhs=agt[:, kc, :],
                            start=(kc == 0), stop=(kc == KC - 1))
                    ot = otp.tile([P, CW], F32, tag="ot", name=f"ot_{qc}_{oc}")
                    nc.vector.tensor_scalar_add(out=ot[:], in0=pp[:],
                                                scalar1=bp_t[:, oc:oc + 1])
                    nc.sync.dma_start(
                        out=out_ext[oc * P:(oc + 1) * P,
                                    qc * CW:(qc + 1) * CW],
                        in_=ot[:])

                for oc in range(4):
                    yield lambda oc=oc: unit(oc)

            # ---- last-chunk projection, staged per collective: A (hp0+hp1,
            # gathered mid-chunk) runs as attention filler; B (hp2) right
            # after attention while the final C (hp3) collective flies; C
            # trails with just 8 matmuls and ring-parallel output DMAs ----
            agt3 = {}
            ota_tiles = {}
            ota2_tiles = {}

            def stage3_A():
                agt = agtp.tile([P, KC, CW], BF16, tag="agt", name="agt_3")
                agt3["t"] = agt
                gp(nc.gpsimd.dma_start(
                    out=agt[:, 0:2, :],
                    in_=ag_out3A[0:2 * P, :].rearrange("(k p) s -> p k s",
                                                       p=P)))
                gp(nc.gpsimd.dma_start(
                    out=agt[:, 4:6, :],
                    in_=ag_out3A[2 * P:4 * P, :].rearrange(
                        "(k p) s -> p k s", p=P)))

            def stage3_B():
                # tail-only: the sync ring is idle here and HWDGE has lower
                # first-byte latency than the SWDGE path.
                agt = agt3["t"]
                nc.sync.dma_start(out=agt[:, 2, :], in_=ag_out3B[0:P, :])
                nc.sync.dma_start(out=agt[:, 6, :], in_=ag_out3B[P:2 * P, :])

            def stage3_C():
                agt = agt3["t"]
                nc.sync.dma_start(out=agt[:, 3, :], in_=ag_out3C[0:P, :])
                nc.sync.dma_start(out=agt[:, 7, :], in_=ag_out3C[P:2 * P, :])

            def proj3A_units():
                def unitA(oc):
                    agt = agt3["t"]
                    pp = psq.tile([P, CW], F32, tag="psq", name=f"ppA_{oc}")
                    for j, kc in enumerate((0, 1, 4, 5)):
                        nc.tensor.matmul(
                            out=pp[:],
                            lhsT=wp_t[:, kc, oc * P:(oc + 1) * P],
                            rhs=agt[:, kc, :],
                            start=(j == 0), stop=(j == 3))
                    ota = otap.tile([P, CW], BF16, tag="ota", name=f"ota_{oc}")
                    ota_tiles[oc] = ota
                    nc.vector.tensor_scalar_add(out=ota[:], in0=pp[:],
                                                scalar1=bp_t[:, oc:oc + 1])

                for oc in range(4):
                    yield lambda oc=oc: unitA(oc)

            def proj3B_units():
                def unitB(oc):
                    agt = agt3["t"]
                    # ps1's pa slots are free at the tail; 3-deep buffering
                    # lets the B matmul groups pipeline past their evictions.
                    pp = ps1.tile([P, CW], F32, tag="pa", name=f"ppB_{oc}")
                    for j, kc in enumerate((2, 6)):
                        nc.tensor.matmul(
                            out=pp[:],
                            lhsT=wp_t[:, kc, oc * P:(oc + 1) * P],
                            rhs=agt[:, kc, :],
                            start=(j == 0), stop=(j == 1))
                    ota2 = otap.tile([P, CW], BF16, tag="ota2",
                                     name=f"ota2_{oc}")
                    ota2_tiles[oc] = ota2
                    nc.vector.tensor_tensor(out=ota2[:], in0=pp[:],
                                            in1=ota_tiles[oc][:], op=ALU.add)

                for oc in range(4):
                    yield lambda oc=oc: unitB(oc)

            def proj3C_units():
                def unitC(oc):
                    agt = agt3["t"]
                    pp = ps1.tile([P, CW], F32, tag="pa", name=f"ppC_{oc}")
                    for j, kc in enumerate((3, 7)):
                        nc.tensor.matmul(
                            out=pp[:],
                            lhsT=wp_t[:, kc, oc * P:(oc + 1) * P],
                            rhs=agt[:, kc, :],
                            start=(j == 0), stop=(j == 1))
                    ot = otp.tile([P, CW], F32, tag="ot", name=f"otC_{oc}")
                    nc.vector.tensor_tensor(out=ot[:], in0=pp[:],
                                            in1=ota2_tiles[oc][:], op=ALU.add)
                    dst = out_ext[oc * P:(oc + 1) * P,
                                  (NQC - 1) * CW:NQC * CW]
                    # spread the four final stores across engine rings so the
                    # kernel tail is one DMA deep, not four.
                    if oc == 0:
                        nc.sync.dma_start(out=dst, in_=ot[:])
                    elif oc == 1:
                        nc.scalar.dma_start(out=dst, in_=ot[:])
                    elif oc == 2:
                        nc.scalar.dma_start(out=dst, in_=ot[:])
                    else:
                        nc.sync.dma_start(out=dst, in_=ot[:])

                for oc in range(4):
                    yield lambda oc=oc: unitC(oc)

            fillers = []
            fillers_late = []

            def pop_filler(late_ok):
                # keep 2 units in reserve: drain_fillers emits them after the
                # chunk's attention so the PE stays busy (and HAM warm) across
                # the chunk boundary while the last pair's softmax normalize
                # chain releases its PSUM slots.
                if len(fillers) > 2:
                    fillers.pop(0)()
                elif late_ok and fillers_late:
                    fillers_late.pop(0)()

            def drain_fillers():
                while fillers:
                    fillers.pop(0)()
                while fillers_late:
                    fillers_late.pop(0)()

            def emit_attention(qc, stage_cb=None, late_cb=None):
                qt = qt_tiles[qc]
                at_tiles = [None] * 4
                at_sets[qc] = at_tiles
                kmax = 4 * (qc + 1)

                for hp in range(4):
                    if hp == 2 and stage_cb is not None:
                        stage_cb()
                    if hp == 3 and late_cb is not None:
                        late_cb()
                    h_e, h_o = 2 * hp, 2 * hp + 1
                    qs_e = qt[0:64, hp, :]
                    qs_o = qt[64:128, hp, :]
                    pa_e = ps1.tile([P, CW], F32, tag="pa",
                                    name=f"pa_{qc}_{hp}_e")
                    pa_o = ps1.tile([P, CW], F32, tag="pa",
                                    name=f"pa_{qc}_{hp}_o")
                    # Keep TWO tiles in flight before flushing: the PE queue
                    # is in-order, so PV(k) — which waits on mask(k) (Vector)
                    # — must not sit immediately behind score(k+1).  With
                    # depth 2 the mask latency hides under the next exp.
                    pending = []

                    # Diagonal k-tiles (kt >= 4qc) only have valid queries at
                    # columns >= 128*(kt-4qc): trim the score matmul, the exp
                    # ACTIVATE, the causal-mask multiplies and the PV matmuls
                    # to that column range.  The skipped columns are exactly
                    # the fully-masked region, so this is bit-equivalent.
                    def toff(kt):
                        return max(0, (kt - 4 * qc)) * P

                    def flush(pending, kmax=kmax, pa_e=pa_e, pa_o=pa_o,
                              h_e=h_e, h_o=h_o, qc=qc):
                        kt, ptile = pending
                        off = toff(kt)
                        if kt >= 4 * qc:
                            # only columns [off, off+128) are partially
                            # masked (the 128x128 triangle); columns beyond
                            # are fully valid.  One op covers both heads.
                            p3 = ptile[:].rearrange("p (h w) -> p h w", w=CW)
                            nc.vector.tensor_mul(
                                out=p3[:, :, off:off + P],
                                in0=p3[:, :, off:off + P],
                                in1=maskd[:])
                        nc.tensor.matmul(
                            out=pa_e[0:VP, off:CW],
                            lhsT=vpad[:, kt, h_e * VP:(h_e + 1) * VP],
                            rhs=ptile[:, off:CW],
                            start=(kt == 0), stop=(kt == kmax - 1))
                        nc.tensor.matmul(
                            out=pa_o[0:VP, off:CW],
                            lhsT=vpad[:, kt, h_o * VP:(h_o + 1) * VP],
                            rhs=ptile[:, CW + off:2 * CW],
                            start=(kt == 0), stop=(kt == kmax - 1))

                    for kt in range(kmax):
                        off = toff(kt)
                        pt = ps3.tile([P, 2 * CW], F32, tag="ps3",
                                      name=f"sc_{qc}_{hp}_{kt}")
                        nc.tensor.matmul(
                            out=pt[:, off:CW],
                            lhsT=k_all[0:64, hp, kt * P:(kt + 1) * P],
                            rhs=qt[0:64, hp, off:CW], start=True, stop=True)
                        nc.tensor.matmul(
                            out=pt[:, CW + off:2 * CW],
                            lhsT=k_all[64:128, hp, kt * P:(kt + 1) * P],
                            rhs=qt[64:128, hp, off:CW], start=True, stop=True)
                        if len(pending) >= 2:
                            flush(pending.pop(0))
                        ptile = ptp.tile([P, 2 * CW], BF16, tag="pt",
                                         name=f"pt_{qc}_{hp}_{kt}")
                        if off:
                            nc.scalar.activation(
                                ptile[:].rearrange("p (h w) -> p h w",
                                                   w=CW)[:, :, off:CW],
                                pt[:].rearrange("p (h w) -> p h w",
                                                w=CW)[:, :, off:CW],
                                AF.Exp, scale=0.125 / 4096.0)
                        else:
                            nc.scalar.activation(ptile[:], pt[:], AF.Exp,
                                                 scale=0.125 / 4096.0)
                        pending.append((kt, ptile))
                        pop_filler(late_ok=(hp == 3 and kt >= kmax - 10))
                    while pending:
                        flush(pending.pop(0))

                    # normalize by the ones-row denominator.  Both heads'
                    # denominators bounce into one [1, 1024] tile so the
                    # reciprocal and the partition-broadcast run once per
                    # head pair instead of twice.
                    at = atp.tile([P, CW], BF16, tag=f"at{hp}",
                                  name=f"at_{qc}_{hp}")
                    at_tiles[hp] = at
                    # the denominator is on PSUM partition 0 (base-0 PSUM
                    # reads are safe for custom-DVE ops; only nonzero base
                    # partitions misread) so the reciprocal reads it without
                    # a bounce copy.
                    rc = smallp.tile([1, 2 * CW], F32, tag="recip",
                                     name=f"rc_{qc}_{hp}")
                    nc.vector.reciprocal_approx_fast(out=rc[:, 0:CW],
                                                     in_=pa_e[0:1, :])
                    nc.vector.reciprocal_approx_fast(out=rc[:, CW:2 * CW],
                                                     in_=pa_o[0:1, :])
                    bc = smallp.tile([64, 2 * CW], F32, tag="bcast",
                                     name=f"bc_{qc}_{hp}")
                    gp(nc.gpsimd.partition_broadcast(bc[:], rc[:]))
                    nc.vector.tensor_tensor(
                        out=at[0:64, :],
                        in0=pa_e[VO:VO + 64, :], in1=bc[:, 0:CW], op=ALU.mult)
                    nc.vector.tensor_tensor(
                        out=at[64:128, :],
                        in0=pa_o[VO:VO + 64, :], in1=bc[:, CW:2 * CW],
                        op=ALU.mult)
                    if debug and qc == 0:
                        nc.sync.dma_start(out=dbg_at[hp, :, :], in_=at[:])
                    # ship this pair's at rows toward the pair AllGather
                    if qc < NQC - 1:
                        nc.sync.dma_start(
                            out=ag_in[qc][hp * P:(hp + 1) * P, :], in_=at[:])
                        if hp == 3:
                            nc.gpsimd.collective_compute(
                                "AllGather", ALU.bypass, replica_groups=RG,
                                ins=[ag_in[qc][:]], outs=[ag_out[qc][:]])
                    elif hp < 2:
                        nc.sync.dma_start(
                            out=ag_in3A[hp * P:(hp + 1) * P, :], in_=at[:])
                        if hp == 1:
                            nc.gpsimd.collective_compute(
                                "AllGather", ALU.bypass, replica_groups=RG,
                                ins=[ag_in3A[:]], outs=[ag_out3A[:]])
                    elif hp == 2:
                        nc.sync.dma_start(out=ag_in3B[:], in_=at[:])
                        nc.gpsimd.collective_compute(
                            "AllGather", ALU.bypass, replica_groups=RG,
                            ins=[ag_in3B[:]], outs=[ag_out3B[:]])
                    else:
                        nc.sync.dma_start(out=ag_in3C[:], in_=at[:])
                        nc.gpsimd.collective_compute(
                            "AllGather", ALU.bypass, replica_groups=RG,
                            ins=[ag_in3C[:]], outs=[ag_out3C[:]])

            # ---- main schedule ----
            # chunk 0 QKV up front; chunk qc+1's QKV rides as PE filler in
            # attention(qc); proj(qc) rides in attention(qc+2) (its pair
            # AllGather completed during attention(qc+1)); proj(2) rides in
            # the second half of attention(3); proj(3) is the tail.
            for u in qkv_units(0, xtr=xtr0, xtrv=xtrv0, pool=ps1, ptag="pa"):
                u()
            if debug:
                nc.sync.dma_start(out=dbg_qt[:], in_=qt_tiles[0][:])
            # chunk-1 QKV also runs up front: its DMAs stream behind chunk
            # 0's on both rings while the PE chews through chunk-0 units, so
            # the start is PE-dense.  Attention chunks then host qkv(qc+2)
            # and the projections concentrate in attention(3), whose causal
            # k-range leaves the most ACT-paced slack to fill.  The upfront
            # units cycle the 3-deep ps1 pool so consecutive units pipeline
            # past their DVE evictions instead of serializing on one bank.
            for u in qkv_units(1, pool=ps1, ptag="pa"):
                u()
            # chunk-2 QKV also runs upfront (its x/weight DMAs stream behind
            # chunks 0-1 while the PE chews through them), so attention(0) —
            # the most Vector-loaded chunk (all-diagonal masks + per-hp
            # normalize every 4 tiles) — carries no filler at all.
            for u in qkv_units(2, pool=ps1, ptag="pa"):
                u()
            held = []
            for qc in range(NQC):
                if qc == 1:
                    units = list(qkv_units(3))
                    fillers.extend(units[:6])
                    held = units[6:]
                if qc == 2:
                    fillers.extend(held)
                    held = []
                    fillers.extend(proj_units(0))
                if qc == NQC - 1:
                    fillers.extend(proj_units(1))
                    fillers_late.extend(proj_units(2))
                    fillers_late.extend(proj3A_units())
                emit_attention(
                    qc,
                    stage_cb=((lambda q=qc: stage_proj(q - 1)) if qc >= 1
                              else None),
                    late_cb=(stage3_A if qc == NQC - 1 else None))
                drain_fillers()
            stage3_B()
            for u in proj3B_units():
                u()
            stage3_C()
            for u in proj3C_units():
                u()
            if debug:
                nc.sync.dma_start(out=dbg_vpad[:], in_=vpad[:])
                nc.sync.dma_start(out=dbg_kall[:], in_=k_all[:])

    nc.finalize()
    return nc


def _get_nc():
    if "nc" not in _CACHE:
        _CACHE["nc"] = _build()
    return _CACHE["nc"]


def _make_mask():
    # M[p, c] = 1.0 iff (c - 384) >= p; pattern pat slice = cols [384-128*pat:][:CW]
    c = np.arange(MW)[None, :]
    p = np.arange(P)[:, None]
    return ((c - 384) >= p).astype(ml_dtypes.bfloat16)


def _dr_interleave(a):
    """[D, N] -> [P, KC2, 2, N] with contraction dim d = c*256 + s*128 + p,
    matching the device-side DoubleRow access pattern."""
    Dd, N = a.shape
    return np.ascontiguousarray(
        a.reshape(KC2, 2, P, N).transpose(2, 0, 1, 3))


def make_in_maps(x, W_attn, b_attn, W_proj, b_proj):
    x = np.asarray(x, np.float32)
    W_attn = np.asarray(W_attn, np.float32)
    b_attn = np.asarray(b_attn, np.float32)
    W_proj = np.asarray(W_proj, np.float32)
    b_proj = np.asarray(b_proj, np.float32)
    mask = _make_mask()
    F8 = ml_dtypes.float8_e4m3fn
    in_maps = []
    for c in range(8):
        b, g = c // 2, c % 2
        sl = slice(g * DL, (g + 1) * DL)
        wqk_c = np.concatenate([W_attn[:, g * DL:(g + 1) * DL],
                                W_attn[:, D + g * DL:D + (g + 1) * DL]],
                               axis=1)
        bqk_c = np.concatenate([b_attn[g * DL:(g + 1) * DL],
                                b_attn[D + g * DL:D + (g + 1) * DL]])
        # q,k weights carry a 64x scale for fp8 range; q,k come out 64x hot,
        # compensated in the exp scale (1/64^2) and the 64x-prescaled bias.
        # The v path stays bf16 (its error feeds the output directly).
        in_maps.append({
            "xT8": _dr_interleave(x[b].T).astype(F8),
            "xTb": np.ascontiguousarray(x[b].T).astype(ml_dtypes.bfloat16),
            "wqkv8": _dr_interleave(wqk_c * 64.0).astype(F8),
            "wv": np.ascontiguousarray(
                W_attn[:, 2 * D + g * DL:2 * D + (g + 1) * DL]
            ).astype(ml_dtypes.bfloat16),
            "bqk": np.ascontiguousarray((64.0 * bqk_c).reshape(8, P).T),
            "bv": b_attn[2 * D + g * DL:2 * D + (g + 1) * DL].reshape(1, DL).copy(),
            "wp": np.ascontiguousarray(W_proj[:, sl]).astype(ml_dtypes.bfloat16),
            "bp": np.ascontiguousarray(b_proj[sl].reshape(4, P).T),
            "maskc": mask,
        })
    return in_maps


def assemble(results):
    out = np.empty((B, S, D), np.float32)
    for c in range(8):
        b, g = c // 2, c % 2
        out[b][:, g * DL:(g + 1) * DL] = results[c]["out"].T
    return out


def kernel(x, W_attn, b_attn, W_proj, b_proj):
    from concourse.bass_utils import run_bass_kernel_spmd
    nc = _get_nc()
    in_maps = make_in_maps(x, W_attn, b_attn, W_proj, b_proj)
    res = run_bass_kernel_spmd(nc, in_maps, core_ids=list(range(8)))
    return assemble(res.results)

